# revision 15
# baseline (speedup 1.0000x reference)
"""Trainium2 Bass kernel for nn_LocalEnergyOpt (molecular-mechanics local energy).

Per batch sample (B=128): features[:, :, 5] packs coords [4096, 3]; col 6 bonds
(i,j,t)x4095; col 7 angles (i,j,k,t)x4094; col 8 torsions (i,j,k,l,t)x4093.
  e_bond = opt[0] * sum k_t (|ci-cj| - r0_t)^2
  e_ang  = opt[1] * sum k_t (theta - th0_t)^2, theta = arccos(clip(cos))
  e_tor  = opt[2] * sum k_t (1 + cos(n_t phi - d_t)), phi = atan2(y, x)
Output [B, 3].

Sharding: pure data parallel, 16 samples per NeuronCore across 8 cores.

Device pipeline per NC (2 waves x 8 samples; GPSIMD Q7 core c handles sample
8w+c on partitions 16c..16c+15):
  stage features flat -> extract packed columns (strided DVE copies; coords
  deinterleaved into x/y/z planes, possible because LPP % 27 == 0) -> dense
  per-sample DRAM scratch -> read back as (a) per-partition-replicated
  x/y/z atom tables, (b) j-wrapped int16 index lists -> ONE ap_gather per
  (list, component) with num_idxs=4096 (the gather cost model charges
  max(table, out) elements per call, so chunking multiplies cost by the
  chunk count) -> SBUF->SBUF diagonal-extract DMA dedups the 16x-replicated
  output into dense [128, 256] tiles -> component-wise DVE/ACT energy
  pipeline -> masked reduce partials -> per-wave PE matmul (one-partition-
  per-group selector) -> [8, 6] -> scale by opt_pars[0:3] -> out.

Gather index order is round-robin across the 16 partitions of a group
(out position i takes the index stored at partition i%16, slot i//16), so
list entry e = 256j + s lands at dense (partition j'=15-only for pads,
u%16 == 15 tail positions); masks are built for exactly those positions.

Torsion angle avoids arccos/atan2 LUTs: cos(phi), sin(phi) are formed by
normalizing (x, y) = (n1.n2, (n1 x b2).n2 / |b2|), and k(1 + cos(n phi - d))
expands to k + A cos(n phi) + B sin(n phi) with per-type A = k cos d,
B = k sin d tables and Chebyshev doubling/tripling for cos/sin(n phi).
"""

import sys
import functools

import numpy as np

sys.path.insert(0, "/opt/trn_rl_repo")

from concourse import bacc, mybir  # noqa: E402
import concourse.tile as tile  # noqa: E402
from concourse.alu_op_type import AluOpType as Op  # noqa: E402

F32 = mybir.dt.float32
I16 = mybir.dt.int16
I32 = mybir.dt.int32
AF = mybir.ActivationFunctionType
AX = mybir.AxisListType

# Problem constants
N_CORES = 8
NS = 16                      # samples per NeuronCore
NB, NA, NT = 4095, 4094, 4093
NATOMS = 4096
MAXLEN = 20465
LPP = 1458                   # flat f32 per partition (multiple of 27)
FLATPAD = 128 * LPP          # 186624 >= 184185
CR = LPP // 9                # 162 packed-column rows per partition
CPP = CR // 3                # 54 coords per partition per component
COLN = 128 * CR              # 20736 dense column length
EPS = 1e-8
PI = float(np.pi)

LIST = 4096                  # per-sample index list length per class (padded)
DP = LIST // 16              # 256 dense positions per partition


def build_nc():
    nc = bacc.Bacc(None, target_bir_lowering=False, debug=False)

    feat = nc.dram_tensor("features", [NS, FLATPAD], F32, kind="ExternalInput")
    bond_t = nc.dram_tensor("bond_type", [15, 2], F32, kind="ExternalInput")
    ang_t = nc.dram_tensor("angle_type", [13, 2], F32, kind="ExternalInput")
    tor_t = nc.dram_tensor("tor_type", [25, 2], F32, kind="ExternalInput")
    mult_f = nc.dram_tensor("mult_f", [1, 25], F32, kind="ExternalInput")
    opt_p = nc.dram_tensor("opt_pars", [1, 47], F32, kind="ExternalInput")
    out_d = nc.dram_tensor("out", [NS, 3], F32, kind="ExternalOutput")

    with tile.TileContext(nc) as tc:
        with (
            tc.tile_pool(name="const", bufs=1) as constp,
            tc.tile_pool(name="stage", bufs=2) as stagep,
            tc.tile_pool(name="cext", bufs=2) as cextp,
            tc.tile_pool(name="table", bufs=1) as tablep,
            tc.tile_pool(name="idxraw", bufs=1) as idxrawp,
            tc.tile_pool(name="idx16", bufs=2) as idx16p,
            tc.tile_pool(name="gath", bufs=3) as gathp,
            tc.tile_pool(name="dense", bufs=16) as densep,
            tc.tile_pool(name="work", bufs=1) as workp,
            tc.tile_pool(name="accp", bufs=1) as accp,
            tc.tile_pool(name="psum", bufs=1, space="PSUM") as psump,
            tc.tile_pool(name="dram", bufs=2, space="DRAM") as dramp,
        ):
            # ---------------- constants -------------------------------------
            # ptab layout: kb[0:15] r0[15:30] ka[30:43] th[43:56]
            #              A[56:81] B[81:106] kt[106:131] nt[131:156]
            cst = constp.tile([128, 932], F32)
            ptab = cst[:, 0:156]
            mB = cst[:, 156:156 + DP]
            mA = cst[:, 412:412 + DP]
            mT = cst[:, 668:668 + DP]
            blk = cst[:, 924:932]
            # fwork layout: row15[0:1] tailf[1:257] colm[257:513] cb[513:517]
            #               opt6[517:523] onep[523:810]
            fwork = constp.tile([128, 812], F32)
            cb = fwork[:, 513:517]
            nc.vector.memset(cb[:, 0:1], EPS)
            nc.vector.memset(cb[:, 1:2], PI / 2.0)
            nc.vector.memset(cb[:, 2:3], 1e-30)
            nc.vector.memset(cb[:, 3:4], -1.0)
            b_eps = cb[:, 0:1]
            b_pi2 = cb[:, 1:2]
            b_tiny = cb[:, 2:3]
            s_neg1 = cb[:, 3:4]

            # parameter tables, assembled on partition 0 then broadcast
            onep = fwork[0:1, 523:810]
            braw = onep[:, 0:30]
            araw = onep[:, 30:56]
            traw = onep[:, 56:106]
            mraw = onep[:, 106:131]
            p156 = onep[:, 131:131 + 156]
            nc.sync.dma_start(out=braw, in_=bond_t.ap().rearrange("a b -> (a b)")[None, :])
            nc.sync.dma_start(out=araw, in_=ang_t.ap().rearrange("a b -> (a b)")[None, :])
            nc.sync.dma_start(out=traw, in_=tor_t.ap().rearrange("a b -> (a b)")[None, :])
            nc.sync.dma_start(out=mraw, in_=mult_f.ap())
            brv = braw.rearrange("p (n d) -> p n d", d=2)
            arv = araw.rearrange("p (n d) -> p n d", d=2)
            trv = traw.rearrange("p (n d) -> p n d", d=2)
            nc.vector.tensor_copy(out=p156[:, 0:15], in_=brv[:, :, 0])      # kb
            nc.vector.tensor_copy(out=p156[:, 15:30], in_=brv[:, :, 1])     # r0
            nc.vector.tensor_copy(out=p156[:, 30:43], in_=arv[:, :, 0])     # ka
            nc.vector.tensor_copy(out=p156[:, 43:56], in_=arv[:, :, 1])     # th0
            # cos d = sin(pi/2 - d); d in [0, 3.15) keeps the arg in [-pi, pi]
            cosd = p156[:, 56:81]
            sind = p156[:, 81:106]
            nc.scalar.activation(cosd, trv[:, :, 1], AF.Sin,
                                 bias=b_pi2[0:1, :], scale=s_neg1[0:1, :])
            nc.scalar.activation(sind, trv[:, :, 1], AF.Sin)
            nc.vector.tensor_mul(out=cosd, in0=cosd, in1=trv[:, :, 0])      # A
            nc.vector.tensor_mul(out=sind, in0=sind, in1=trv[:, :, 0])      # B
            nc.vector.tensor_copy(out=p156[:, 106:131], in_=trv[:, :, 0])   # kt
            nc.vector.tensor_copy(out=p156[:, 131:156], in_=mraw)           # nt
            p_dram = dramp.tile([1, 156], F32)
            nc.sync.dma_start(out=p_dram[:], in_=p156)
            nc.sync.dma_start(out=ptab, in_=p_dram[:].to_broadcast([128, 156]))

            # masks: pad list entries land on partitions p%16==15 at dense
            # cols u with u%16==15 and u >= 256-16*ntail (round-robin unwrap)
            iwork = constp.tile([128, 524], I32)
            pidx = iwork[:, 0:1]
            colx = iwork[:, 1:257]
            and15 = iwork[:, 257:258]
            r15i = iwork[:, 258:259]
            blki = iwork[:, 259:267]
            pdiv = iwork[:, 267:268]
            cm_i = iwork[:, 268:524]
            row15 = fwork[:, 0:1]
            tailf = fwork[:, 1:257]
            colm = fwork[:, 257:513]
            nc.gpsimd.iota(pidx, pattern=[[1, 1]], base=0, channel_multiplier=1)
            nc.gpsimd.iota(colx, pattern=[[1, 256]], base=0, channel_multiplier=0)
            nc.vector.tensor_scalar(out=and15, in0=pidx, scalar1=15, scalar2=None,
                                    op0=Op.bitwise_and)
            nc.vector.tensor_scalar(out=r15i, in0=and15, scalar1=15, scalar2=None,
                                    op0=Op.is_equal)
            nc.vector.tensor_copy(out=row15, in_=r15i)
            for msk, ntail in ((mB, 1), (mA, 2), (mT, 3)):
                nc.vector.tensor_scalar(out=tailf, in0=colx, scalar1=DP - 16 * ntail,
                                        scalar2=None, op0=Op.is_ge)
                nc.vector.tensor_tensor(out=msk, in0=tailf,
                                        in1=row15.to_broadcast([128, DP]), op=Op.mult)
            # fold in the u%16==15 condition (shared across the three masks)
            nc.vector.tensor_scalar(out=cm_i, in0=colx, scalar1=15, scalar2=None,
                                    op0=Op.bitwise_and)
            nc.vector.tensor_scalar(out=colm, in0=cm_i, scalar1=15,
                                    scalar2=None, op0=Op.is_equal)
            for msk in (mB, mA, mT):
                nc.vector.tensor_tensor(out=msk, in0=msk, in1=colm, op=Op.mult)
                nc.vector.tensor_scalar(out=msk, in0=msk, scalar1=-1.0, scalar2=1.0,
                                        op0=Op.mult, op1=Op.add)
            # selector: blk[p, c] = 1 iff p//16 == c -> PE sums each 16-part
            # group (the dedup slices are disjoint partials) into PSUM row c
            nc.vector.tensor_scalar(out=pdiv, in0=pidx, scalar1=4, scalar2=None,
                                    op0=Op.arith_shift_right)
            nc.gpsimd.iota(blki, pattern=[[1, 8]], base=0, channel_multiplier=0)
            nc.vector.tensor_tensor(out=blki, in0=pdiv.to_broadcast([128, 8]),
                                    in1=blki, op=Op.is_equal)
            nc.vector.tensor_copy(out=blk, in_=blki)

            accb = accp.tile([128, DP + 6 + 8], F32)
            scr = accb[:, 0:DP]
            acc6 = accb[:, DP:DP + 6]
            otmp = accb[0:8, DP + 6:DP + 12]
            rtmp = accb[:, DP + 12:DP + 13]
            opt6 = fwork[0:8, 517:523]
            nc.sync.dma_start(
                out=opt6,
                in_=opt_p.ap()[:, 0:3][:, None, :].to_broadcast([8, 2, 3]),
            )

            def ptab_view(lo, n):
                return ptab[:, lo:lo + n].rearrange("p (n d) -> p n d", d=1)

            kb_t = ptab_view(0, 15)
            r0_t = ptab_view(15, 15)
            ka_t = ptab_view(30, 13)
            th_t = ptab_view(43, 13)
            A_t = ptab_view(56, 25)
            B_t = ptab_view(81, 25)
            kt_t = ptab_view(106, 25)
            nt_t = ptab_view(131, 25)

            for w in range(2):
                # ------------- stage + column extraction -------------------
                # coord planes [3 comps, 8 samples, 128*54]; class planes
                # [3 classes, 8 samples, 128*162]
                cpl = dramp.tile([3, 8, 128 * CPP], F32, tag="cpl")
                clp = dramp.tile([3, 8, COLN], F32, tag="clp")
                for s8 in range(8):
                    s = 8 * w + s8
                    stage = stagep.tile([128, LPP], F32, tag="stage")
                    nc.sync.dma_start(
                        out=stage[:], in_=feat.ap()[s].rearrange("(p f) -> p f", f=LPP)
                    )
                    cd = cextp.tile([128, 3 * CPP + 3 * CR], F32, tag="cd")
                    st27 = stage[:].rearrange("p (u t) -> p u t", t=27)
                    for m in range(3):
                        nc.vector.tensor_copy(
                            out=cd[:, CPP * m:CPP * (m + 1)],
                            in_=st27[:, :, 9 * m + 5])
                    stv = stage[:].rearrange("p (r n) -> p r n", n=9)
                    nc.vector.tensor_copy(
                        out=cd[:, 3 * CPP:].rearrange("p (k r) -> p k r", r=CR),
                        in_=stv[:, :, 6:9].rearrange("p r c -> p c r"),
                    )
                    # partition dim FIRST in both APs: the DMA cost model
                    # divides total bytes by the first AP dim's count
                    nc.sync.dma_start(
                        out=cpl[:][:, s8].rearrange("m (p u) -> p m u", u=CPP),
                        in_=cd[:, 0:3 * CPP].rearrange("p (m u) -> p m u", u=CPP),
                    )
                    nc.sync.dma_start(
                        out=clp[:][:, s8].rearrange("k (p r) -> p k r", r=CR),
                        in_=cd[:, 3 * CPP:].rearrange("p (k r) -> p k r", r=CR),
                    )

                # ------------- gather tables (replicated coords) -----------
                # partition 16c+j holds sample c's full per-component table
                tabs = []
                for m in range(3):
                    tb = tablep.tile([128, NATOMS], F32, tag=f"tab{m}")
                    nc.sync.dma_start(
                        out=tb[:],
                        in_=cpl[:][m][:, None, 0:NATOMS].to_broadcast(
                            [8, 16, NATOMS]),
                    )
                    tabs.append(tb[:].rearrange("p (n d) -> p n d", d=1))
                xt, yt, zt = tabs

                # ------------- index readback + int16 conversion -----------
                iraw = idxrawp.tile([128, 3072], F32, tag="iraw")
                braw_i = iraw[:, 0:768]
                araw_i = iraw[:, 768:1792]
                traw_i = iraw[:, 1792:3072]
                nc.sync.dma_start(
                    out=braw_i,
                    in_=clp[:][0][:, 0:12288].rearrange("s (j f) -> s j f", f=768))
                nc.sync.dma_start(
                    out=araw_i,
                    in_=clp[:][1][:, 0:16384].rearrange("s (j f) -> s j f", f=1024))
                nc.sync.dma_start(
                    out=traw_i,
                    in_=clp[:][2][:, 0:20480].rearrange("s (j f) -> s j f", f=1280))

                idxt = idx16p.tile([128, 12 * 256], I16, tag="idxt")

                def idx_list(n):
                    return idxt[:, 256 * n:256 * (n + 1)]

                bv = braw_i.rearrange("p (e k) -> p e k", k=3)
                av = araw_i.rearrange("p (e k) -> p e k", k=4)
                tv = traw_i.rearrange("p (e k) -> p e k", k=5)
                for k in range(3):
                    nc.vector.tensor_copy(out=idx_list(k), in_=bv[:, :, k])
                for k in range(4):
                    nc.vector.tensor_copy(out=idx_list(3 + k), in_=av[:, :, k])
                for k in range(5):
                    nc.vector.tensor_copy(out=idx_list(7 + k), in_=tv[:, :, k])
                # lists: 0,1,2 = bond i,j,t; 3..6 = angle i,j,k,t; 7..11 = tor i,j,k,l,t

                def gd(idx_n, tab_ap, n_elems):
                    """One-shot ap_gather of a full 4096-index list, then
                    dedup the 16x-replicated output (keep each group
                    leader's row) into a dense [128, 256] tile via a
                    partition-diagonal SBUF->SBUF DMA."""
                    g = gathp.tile([128, LIST], F32, tag="g")
                    nc.gpsimd.ap_gather(
                        out_ap=g[:].rearrange("p (n d) -> p n d", d=1),
                        in_ap=tab_ap,
                        idxs_ap=idx_list(idx_n),
                        channels=128,
                        num_elems=n_elems,
                        d=1,
                        num_idxs=LIST,
                    )
                    dn = densep.tile([128, DP], F32, tag="dn")
                    nc.sync.dma_start(
                        out=dn[:],
                        in_=g[:].rearrange("(c j) f -> c j f", j=16)[:, 0, :]
                            .rearrange("c (j u) -> c j u", u=DP),
                    )
                    return dn

                acc = acc6[:, 3 * w:3 * w + 3]
                nc.vector.memset(acc, 0.0)

                # ==================== BONDS ====================
                xi, yi, zi = gd(0, xt, NATOMS), gd(0, yt, NATOMS), gd(0, zt, NATOMS)
                xj, yj, zj = gd(1, xt, NATOMS), gd(1, yt, NATOMS), gd(1, zt, NATOMS)
                kb_g = gd(2, kb_t, 15)
                r0_g = gd(2, r0_t, 15)
                wb = workp.tile([128, 8 * DP], F32, tag="w8")
                dx = wb[:, 0:DP]
                dy = wb[:, 1 * DP:2 * DP]
                dz = wb[:, 2 * DP:3 * DP]
                nc.vector.tensor_sub(out=dx, in0=xi[:], in1=xj[:])
                nc.vector.tensor_sub(out=dy, in0=yi[:], in1=yj[:])
                nc.vector.tensor_sub(out=dz, in0=zi[:], in1=zj[:])
                sx = wb[:, 3 * DP:4 * DP]
                sy = wb[:, 4 * DP:5 * DP]
                nc.scalar.activation(sx, dx, AF.Square)
                nc.scalar.activation(sy, dy, AF.Square)
                r2 = wb[:, 5 * DP:6 * DP]
                nc.vector.tensor_mul(out=dz, in0=dz, in1=dz)
                nc.vector.tensor_add(out=r2, in0=sx, in1=sy)
                nc.vector.tensor_add(out=r2, in0=r2, in1=dz)
                r = wb[:, 0:DP]                        # dx dead
                nc.scalar.activation(r, r2, AF.Sqrt, bias=b_eps)
                u = wb[:, 1 * DP:2 * DP]               # dy dead
                nc.vector.tensor_sub(out=u, in0=r, in1=r0_g[:])
                e = wb[:, 2 * DP:3 * DP]               # dz dead
                nc.scalar.activation(e, u, AF.Square)
                km = wb[:, 3 * DP:4 * DP]              # sx dead
                nc.vector.tensor_tensor(out=km, in0=kb_g[:], in1=mB, op=Op.mult)
                nc.vector.tensor_mul(out=scr, in0=e, in1=km)
                nc.vector.tensor_reduce(out=rtmp, in_=scr, axis=AX.X, op=Op.add)
                nc.vector.tensor_add(out=acc[:, 0:1], in0=acc[:, 0:1], in1=rtmp)

                # ==================== ANGLES ====================
                gxi, gyi, gzi = gd(3, xt, NATOMS), gd(3, yt, NATOMS), gd(3, zt, NATOMS)
                gxj, gyj, gzj = gd(4, xt, NATOMS), gd(4, yt, NATOMS), gd(4, zt, NATOMS)
                gxk, gyk, gzk = gd(5, xt, NATOMS), gd(5, yt, NATOMS), gd(5, zt, NATOMS)
                ka_g = gd(6, ka_t, 13)
                th_g = gd(6, th_t, 13)
                v6 = workp.tile([128, 6 * DP], F32, tag="w6")
                v1x, v1y, v1z = v6[:, 0:DP], v6[:, DP:2 * DP], v6[:, 2 * DP:3 * DP]
                v2x, v2y, v2z = (v6[:, 3 * DP:4 * DP], v6[:, 4 * DP:5 * DP],
                                 v6[:, 5 * DP:6 * DP])
                nc.vector.tensor_sub(out=v1x, in0=gxi[:], in1=gxj[:])
                nc.vector.tensor_sub(out=v1y, in0=gyi[:], in1=gyj[:])
                nc.vector.tensor_sub(out=v1z, in0=gzi[:], in1=gzj[:])
                nc.vector.tensor_sub(out=v2x, in0=gxk[:], in1=gxj[:])
                nc.vector.tensor_sub(out=v2y, in0=gyk[:], in1=gyj[:])
                nc.vector.tensor_sub(out=v2z, in0=gzk[:], in1=gzj[:])
                wa = workp.tile([128, 8 * DP], F32, tag="w8")
                t9 = workp.tile([128, 2 * DP], F32, tag="w2")
                t0 = t9[:, 0:DP]
                d11 = wa[:, 0:DP]
                d22 = wa[:, 1 * DP:2 * DP]
                d12 = wa[:, 2 * DP:3 * DP]

                def dot3c(dst, scr_, ax, ay, az, bx, by, bz):
                    nc.vector.tensor_mul(out=dst, in0=ax, in1=bx)
                    nc.vector.tensor_mul(out=scr_, in0=ay, in1=by)
                    nc.vector.tensor_add(out=dst, in0=dst, in1=scr_)
                    nc.vector.tensor_mul(out=scr_, in0=az, in1=bz)
                    nc.vector.tensor_add(out=dst, in0=dst, in1=scr_)

                dot3c(d11, t0, v1x, v1y, v1z, v1x, v1y, v1z)
                dot3c(d22, t0, v2x, v2y, v2z, v2x, v2y, v2z)
                dot3c(d12, t0, v1x, v1y, v1z, v2x, v2y, v2z)
                s1 = wa[:, 3 * DP:4 * DP]
                s2a = wa[:, 4 * DP:5 * DP]
                nc.scalar.activation(s1, d11, AF.Sqrt, bias=b_eps)
                nc.scalar.activation(s2a, d22, AF.Sqrt, bias=b_eps)
                den = wa[:, 5 * DP:6 * DP]
                nc.vector.tensor_mul(out=den, in0=s1, in1=s2a)
                cosv = wa[:, 6 * DP:7 * DP]
                nc.vector.reciprocal(out=den, in_=den)
                nc.vector.tensor_mul(out=cosv, in0=d12, in1=den)
                cosc = wa[:, 7 * DP:8 * DP]
                nc.vector.tensor_scalar(
                    out=cosc, in0=cosv, scalar1=-1.0 + 1e-6, scalar2=1.0 - 1e-6,
                    op0=Op.max, op1=Op.min,
                )
                # theta = arccos(cosc) via two bounded-arg arctan branches
                # (ACT Arctan domain is [-pi/2, pi/2] so |arg| <= 1 required):
                #  |c| >  s: theta = arctan(s/c) + pi*(c<0)
                #  |c| <= s: theta = pi/2 - arctan(c/s), s = sqrt(1-c^2)
                cc = wa[:, 0:DP]                       # d11 dead
                nc.scalar.activation(cc, cosc, AF.Square)
                om = wa[:, 1 * DP:2 * DP]              # d22 dead
                nc.vector.tensor_scalar(
                    out=om, in0=cc, scalar1=-1.0, scalar2=1.0, op0=Op.mult, op1=Op.add
                )
                sn = wa[:, 2 * DP:3 * DP]              # d12 dead
                nc.scalar.activation(sn, om, AF.Sqrt)
                sgn = wa[:, 3 * DP:4 * DP]             # s1 dead
                nc.vector.tensor_scalar(
                    out=sgn, in0=cosc, scalar1=0.0, scalar2=None, op0=Op.is_ge)
                nc.vector.tensor_scalar(
                    out=sgn, in0=sgn, scalar1=2e-18, scalar2=-1e-18,
                    op0=Op.mult, op1=Op.add)
                csafe = wa[:, 4 * DP:5 * DP]           # s2a dead
                nc.vector.tensor_add(out=csafe, in0=cosc, in1=sgn)
                ra = wa[:, 3 * DP:4 * DP]              # sgn dead
                nc.vector.reciprocal(out=csafe, in_=csafe)
                nc.vector.tensor_mul(out=ra, in0=sn, in1=csafe)
                nc.vector.tensor_scalar(
                    out=ra, in0=ra, scalar1=-1.0, scalar2=1.0, op0=Op.max, op1=Op.min)
                ata = wa[:, 4 * DP:5 * DP]             # csafe dead
                nc.scalar.activation(ata, ra, AF.Arctan)
                corr = wa[:, 5 * DP:6 * DP]            # den dead
                nc.vector.tensor_scalar(
                    out=corr, in0=cosc, scalar1=0.0, scalar2=PI, op0=Op.is_lt, op1=Op.mult
                )
                tha = wa[:, 3 * DP:4 * DP]             # ra dead
                nc.vector.tensor_add(out=tha, in0=ata, in1=corr)
                rb = wa[:, 4 * DP:5 * DP]              # ata dead
                nc.vector.reciprocal(out=sn, in_=sn)
                nc.vector.tensor_mul(out=rb, in0=cosc, in1=sn)
                nc.vector.tensor_scalar(
                    out=rb, in0=rb, scalar1=-1.0, scalar2=1.0, op0=Op.max, op1=Op.min)
                thb = wa[:, 5 * DP:6 * DP]             # corr dead
                nc.scalar.activation(thb, rb, AF.Arctan)
                nc.vector.tensor_scalar(
                    out=thb, in0=thb, scalar1=-1.0, scalar2=PI / 2.0,
                    op0=Op.mult, op1=Op.add)
                wi = workp.tile([128, 2 * DP], I32, tag="wi")
                mbr = wi[:, 0:DP]
                nc.vector.tensor_scalar(
                    out=mbr, in0=cc, scalar1=0.5, scalar2=None, op0=Op.is_gt)
                th = wa[:, 6 * DP:7 * DP]              # cosv dead
                nc.vector.select(out=th, mask=mbr, on_true=tha, on_false=thb)
                ua = wa[:, 0:DP]                       # cc dead
                nc.vector.tensor_sub(out=ua, in0=th, in1=th_g[:])
                ea = wa[:, 1 * DP:2 * DP]              # om dead
                nc.scalar.activation(ea, ua, AF.Square)
                kma = wa[:, 2 * DP:3 * DP]             # sn dead
                nc.vector.tensor_tensor(out=kma, in0=ka_g[:], in1=mA, op=Op.mult)
                nc.vector.tensor_mul(out=scr, in0=ea, in1=kma)
                nc.vector.tensor_reduce(out=rtmp, in_=scr, axis=AX.X, op=Op.add)
                nc.vector.tensor_add(out=acc[:, 1:2], in0=acc[:, 1:2], in1=rtmp)

                # ==================== TORSIONS ====================
                txi, tyi, tzi = gd(7, xt, NATOMS), gd(7, yt, NATOMS), gd(7, zt, NATOMS)
                txj, tyj, tzj = gd(8, xt, NATOMS), gd(8, yt, NATOMS), gd(8, zt, NATOMS)
                txk, tyk, tzk = gd(9, xt, NATOMS), gd(9, yt, NATOMS), gd(9, zt, NATOMS)
                txl, tyl, tzl = gd(10, xt, NATOMS), gd(10, yt, NATOMS), gd(10, zt, NATOMS)
                A_g = gd(11, A_t, 25)
                B_g = gd(11, B_t, 25)
                kt_g = gd(11, kt_t, 25)
                nt_g = gd(11, nt_t, 25)
                b9 = workp.tile([128, 9 * DP], F32, tag="w9")

                def b(n):
                    return b9[:, DP * n:DP * (n + 1)]
                # b(0..2) = b1 xyz; b(3..5) = b2 xyz; b(6..8) = b3 xyz
                nc.vector.tensor_sub(out=b(0), in0=txj[:], in1=txi[:])
                nc.vector.tensor_sub(out=b(1), in0=tyj[:], in1=tyi[:])
                nc.vector.tensor_sub(out=b(2), in0=tzj[:], in1=tzi[:])
                nc.vector.tensor_sub(out=b(3), in0=txk[:], in1=txj[:])
                nc.vector.tensor_sub(out=b(4), in0=tyk[:], in1=tyj[:])
                nc.vector.tensor_sub(out=b(5), in0=tzk[:], in1=tzj[:])
                nc.vector.tensor_sub(out=b(6), in0=txl[:], in1=txk[:])
                nc.vector.tensor_sub(out=b(7), in0=tyl[:], in1=tyk[:])
                nc.vector.tensor_sub(out=b(8), in0=tzl[:], in1=tzk[:])
                # n1 = b1 x b2 -> cr 0..2 ; n2 = b2 x b3 -> cr 3..5
                cr_ = workp.tile([128, 6 * DP], F32, tag="w6b")

                def crv(n):
                    return cr_[:, DP * n:DP * (n + 1)]

                tmp = workp.tile([128, 2 * DP], F32, tag="w2")
                t0_ = tmp[:, 0:DP]
                t1_ = tmp[:, DP:2 * DP]
                for m in range(3):
                    mp1, mp2 = (m + 1) % 3, (m + 2) % 3
                    nc.vector.tensor_mul(out=t0_, in0=b(0 + mp1), in1=b(3 + mp2))
                    nc.vector.tensor_mul(out=t1_, in0=b(0 + mp2), in1=b(3 + mp1))
                    nc.vector.tensor_sub(out=crv(m), in0=t0_, in1=t1_)
                    nc.vector.tensor_mul(out=t0_, in0=b(3 + mp1), in1=b(6 + mp2))
                    nc.vector.tensor_mul(out=t1_, in0=b(3 + mp2), in1=b(6 + mp1))
                    nc.vector.tensor_sub(out=crv(3 + m), in0=t0_, in1=t1_)
                wt = workp.tile([128, 8 * DP], F32, tag="w8")
                q2 = wt[:, 0:DP]
                dot3c(q2, t0_, b(3), b(4), b(5), b(3), b(4), b(5))
                # m1' = n1 x b2 (normalization folded into rn) -> b(6..8)
                # (b3 planes dead after the cross products)
                for m in range(3):
                    mp1, mp2 = (m + 1) % 3, (m + 2) % 3
                    nc.vector.tensor_mul(out=t0_, in0=crv(mp1), in1=b(3 + mp2))
                    nc.vector.tensor_mul(out=t1_, in0=crv(mp2), in1=b(3 + mp1))
                    nc.vector.tensor_sub(out=b(6 + m), in0=t0_, in1=t1_)
                X = wt[:, 1 * DP:2 * DP]
                Y = wt[:, 2 * DP:3 * DP]
                dot3c(X, t0_, crv(0), crv(1), crv(2), crv(3), crv(4), crv(5))
                dot3c(Y, t0_, b(6), b(7), b(8), crv(3), crv(4), crv(5))
                rn = wt[:, 3 * DP:4 * DP]
                nc.scalar.activation(rn, q2, AF.Sqrt, bias=b_eps)
                y = wt[:, 4 * DP:5 * DP]
                nc.vector.reciprocal(out=rn, in_=rn)
                nc.vector.tensor_mul(out=y, in0=Y, in1=rn)
                hx = wt[:, 5 * DP:6 * DP]
                hy = wt[:, 6 * DP:7 * DP]
                nc.scalar.activation(hx, X, AF.Square)
                nc.scalar.activation(hy, y, AF.Square)
                h = wt[:, 7 * DP:8 * DP]
                nc.vector.tensor_add(out=h, in0=hx, in1=hy)
                rh = wt[:, 5 * DP:6 * DP]              # hx dead
                nc.scalar.activation(rh, h, AF.Sqrt, bias=b_tiny)
                c = wt[:, 0:DP]                        # q2 dead
                s = wt[:, 6 * DP:7 * DP]               # hy dead
                nc.vector.reciprocal(out=rh, in_=rh)
                nc.vector.tensor_mul(out=c, in0=X, in1=rh)
                nc.vector.tensor_mul(out=s, in0=y, in1=rh)
                # Chebyshev: cos/sin of 2phi and 3phi (b1/b2 planes dead)
                cc_ = b(0)
                c2 = b(1)
                s2 = b(2)
                c3 = b(3)
                s3 = b(4)
                sc = b(5)
                nc.scalar.activation(cc_, c, AF.Square)
                nc.vector.tensor_scalar(
                    out=c2, in0=cc_, scalar1=2.0, scalar2=-1.0, op0=Op.mult, op1=Op.add)
                nc.vector.tensor_mul(out=sc, in0=s, in1=c)
                nc.vector.tensor_scalar(
                    out=s2, in0=sc, scalar1=2.0, scalar2=None, op0=Op.mult)
                nc.vector.tensor_scalar(
                    out=t0_, in0=cc_, scalar1=4.0, scalar2=-3.0, op0=Op.mult, op1=Op.add)
                nc.vector.tensor_mul(out=c3, in0=t0_, in1=c)
                nc.vector.tensor_scalar(
                    out=t0_, in0=cc_, scalar1=4.0, scalar2=-1.0, op0=Op.mult, op1=Op.add)
                nc.vector.tensor_mul(out=s3, in0=t0_, in1=s)
                wi2 = workp.tile([128, 2 * DP], I32, tag="wi")
                m2m = wi2[:, 0:DP]
                m3m = wi2[:, DP:2 * DP]
                nc.vector.tensor_scalar(
                    out=m2m, in0=nt_g[:], scalar1=2.0, scalar2=None, op0=Op.is_equal)
                nc.vector.tensor_scalar(
                    out=m3m, in0=nt_g[:], scalar1=3.0, scalar2=None, op0=Op.is_equal)
                cn = wt[:, 3 * DP:4 * DP]              # rn dead
                sn2 = wt[:, 4 * DP:5 * DP]             # y dead
                nc.vector.select(out=cn, mask=m2m, on_true=c2, on_false=c)
                nc.vector.select(out=cn, mask=m3m, on_true=c3, on_false=cn)
                nc.vector.select(out=sn2, mask=m2m, on_true=s2, on_false=s)
                nc.vector.select(out=sn2, mask=m3m, on_true=s3, on_false=sn2)
                tt1 = wt[:, 5 * DP:6 * DP]             # rh dead
                tt2 = wt[:, 6 * DP:7 * DP]             # s dead (selects done)
                nc.vector.tensor_mul(out=tt1, in0=cn, in1=A_g[:])
                nc.vector.tensor_mul(out=tt2, in0=sn2, in1=B_g[:])
                esum = wt[:, 7 * DP:8 * DP]            # h dead
                nc.vector.tensor_add(out=esum, in0=tt1, in1=tt2)
                nc.vector.tensor_add(out=esum, in0=esum, in1=kt_g[:])
                nc.vector.tensor_mul(out=scr, in0=esum, in1=mT)
                nc.vector.tensor_reduce(out=rtmp, in_=scr, axis=AX.X, op=Op.add)
                nc.vector.tensor_add(out=acc[:, 2:3], in0=acc[:, 2:3], in1=rtmp)

            # ------------- final reduction: [128, 6] -> [8, 6] -> out ------
            pacc = psump.tile([8, 6], F32, tag="pacc")
            nc.tensor.matmul(out=pacc[:], lhsT=blk, rhs=acc6, start=True, stop=True)
            nc.vector.tensor_copy(out=otmp, in_=pacc[:])
            nc.vector.tensor_mul(out=otmp, in0=otmp, in1=opt6)
            nc.sync.dma_start(out=out_d.ap()[0:8, :], in_=otmp[:, 0:3])
            nc.sync.dma_start(out=out_d.ap()[8:16, :], in_=otmp[:, 3:6])

    nc.compile()
    return nc


@functools.lru_cache(maxsize=1)
def _get_nc():
    return build_nc()


def make_in_maps(inputs):
    """Shard full inputs into 8 per-core input maps."""
    feats = np.ascontiguousarray(inputs["features"], dtype=np.float32)
    Bf = feats.shape[0]
    flat = feats.reshape(Bf, -1)
    flat = np.concatenate(
        [flat, np.zeros((Bf, FLATPAD - flat.shape[1]), np.float32)], axis=1
    )
    bond_type = np.ascontiguousarray(inputs["bond_type"], np.float32)
    angle_type = np.ascontiguousarray(inputs["angle_type"], np.float32)
    tor_type = np.ascontiguousarray(inputs["tor_type"], np.float32)
    mult_f = np.ascontiguousarray(inputs["multiplicity"], np.float32).reshape(1, 25)
    opt = np.ascontiguousarray(inputs["opt_pars"], np.float32).reshape(1, 47)
    n_nc = Bf // NS
    in_maps = []
    for k in range(n_nc):
        in_maps.append({
            "features": flat[NS * k:NS * (k + 1)],
            "bond_type": bond_type,
            "angle_type": angle_type,
            "tor_type": tor_type,
            "mult_f": mult_f,
            "opt_pars": opt,
        })
    return in_maps


def kernel(**inputs) -> np.ndarray:
    from concourse.bass_utils import run_bass_kernel_spmd

    nc = _get_nc()
    in_maps = make_in_maps(inputs)
    res = run_bass_kernel_spmd(nc, in_maps, core_ids=list(range(len(in_maps))))
    outs = [res.results[k]["out"] for k in range(len(in_maps))]
    return np.concatenate(outs, axis=0).astype(np.float32)


def simulate_one_core(inputs, nc=None):
    """CoreSim a single NC on the first 16 samples (for correctness dev)."""
    import concourse.bass_interp as bass_interp

    if nc is None:
        nc = _get_nc()
    in_map = make_in_maps(inputs)[0]
    sim = bass_interp.MultiCoreSim(nc, 1)
    for name, val in in_map.items():
        sim.cores[0].tensor(name)[:] = val
    sim.simulate(check_with_hw=False)
    return np.array(sim.cores[0].mem_tensor("out"))


if __name__ == "__main__":
    nc = build_nc()
    print("build ok")


# revision 20
# speedup vs baseline: 1.9674x; 1.9674x over previous
"""Trainium2 Bass kernel for nn_LocalEnergyOpt (molecular-mechanics local energy).

Per batch sample (B=128): features[:, :, 5] packs coords [4096, 3]; col 6 bonds
(i,j,t)x4095; col 7 angles (i,j,k,t)x4094; col 8 torsions (i,j,k,l,t)x4093.
  e_bond = opt[0] * sum k_t (|ci-cj| - r0_t)^2
  e_ang  = opt[1] * sum k_t (theta - th0_t)^2, theta = arccos(clip(cos))
  e_tor  = opt[2] * sum k_t (1 + cos(n_t phi - d_t)), phi = atan2(y, x)
Output [B, 3].

Sharding: pure data parallel, 16 samples per NeuronCore across 8 cores.

Device pipeline per NC (2 waves x 8 samples; GPSIMD Q7 core c handles sample
8w+c on partitions 16c..16c+15):
  stage features flat (LPP=1458 keeps per-partition coord phase uniform) ->
  extract packed columns (strided DVE copies; coords converted to bf16 and
  re-spaced into 4-wide padded atom rows) -> dense per-sample DRAM scratch ->
  per-partition-replicated bf16 coord table + j-wrapped int16 index lists ->
  ONE ap_gather per list (num_idxs=4096): the Q7 gather ucode cost is
  num_idxs * (a + b*words), so bf16 d=4 rows (2 words) cost ~2/3 of f32
  d=3 (3 words) and one-shot drops 8x512 chunking overheads -> SBUF->SBUF
  partition-diagonal DMA dedups the 16x-replicated output into dense
  [128, 256, d] tiles -> f32 DVE/ACT energy pipeline (pad lane is zero so
  d=4 reductions are exact) -> masked reduce partials -> per-wave PE
  matmul (one-partition-per-group selector) -> [8, 6] -> scale by
  opt_pars[0:3] -> out.

One-shot gather index order is round-robin within each 16-partition group
(out position i takes the index from partition i%16, slot i//16), so pad
list entries land at dense (p%16==15, u%16==15, u >= 256-16*ntail); masks
are built for exactly those positions.

Torsion angle avoids arccos/atan2 LUTs: cos(phi), sin(phi) are formed by
normalizing (x, y) = (n1.n2, (n1 x b2).n2 / |b2|), and cos(n phi - d)
expands via Chebyshev doubling/tripling + per-type (cos d, sin d) tables.
"""

import sys
import functools

import numpy as np

sys.path.insert(0, "/opt/trn_rl_repo")

from concourse import bacc, mybir  # noqa: E402
import concourse.tile as tile  # noqa: E402
from concourse.alu_op_type import AluOpType as Op  # noqa: E402

F32 = mybir.dt.float32
BF16 = mybir.dt.bfloat16
I16 = mybir.dt.int16
I32 = mybir.dt.int32
AF = mybir.ActivationFunctionType
AX = mybir.AxisListType

# Problem constants
N_CORES = 8
NS = 16                      # samples per NeuronCore
NB, NA, NT = 4095, 4094, 4093
NATOMS = 4096
MAXLEN = 20465
LPP = 1458                   # flat f32 per partition (multiple of 27)
FLATPAD = 128 * LPP          # 186624 >= 184185
CR = LPP // 9                # 162 packed-column rows per partition
CPP = CR // 3                # 54 complete atoms per partition (coords col)
COLN = 128 * CR              # 20736 dense column length
EPS = 1e-8
PI = float(np.pi)

LIST = 4096                  # per-core index list length per class (padded)
DP = LIST // 16              # 256 dense positions per partition


def build_nc():
    nc = bacc.Bacc(None, target_bir_lowering=False, debug=False)

    feat = nc.dram_tensor("features", [NS, FLATPAD], F32, kind="ExternalInput")
    bond_t = nc.dram_tensor("bond_type", [15, 2], F32, kind="ExternalInput")
    ang_t = nc.dram_tensor("angle_type", [13, 2], F32, kind="ExternalInput")
    tor_t = nc.dram_tensor("tor_type", [25, 2], F32, kind="ExternalInput")
    mult_f = nc.dram_tensor("mult_f", [1, 25], F32, kind="ExternalInput")
    opt_p = nc.dram_tensor("opt_pars", [1, 47], F32, kind="ExternalInput")
    out_d = nc.dram_tensor("out", [NS, 3], F32, kind="ExternalOutput")

    with tile.TileContext(nc) as tc:
        with (
            tc.tile_pool(name="const", bufs=1) as constp,
            tc.tile_pool(name="stage", bufs=2) as stagep,
            tc.tile_pool(name="cext", bufs=2) as cextp,
            tc.tile_pool(name="table", bufs=1) as tablep,
            tc.tile_pool(name="idxraw", bufs=1) as idxrawp,
            tc.tile_pool(name="idx16", bufs=2) as idx16p,
            tc.tile_pool(name="gath", bufs=2) as gathp,
            tc.tile_pool(name="dense", bufs=8) as densep,
            tc.tile_pool(name="work", bufs=1) as workp,
            tc.tile_pool(name="accp", bufs=1) as accp,
            tc.tile_pool(name="psum", bufs=1, space="PSUM") as psump,
            tc.tile_pool(name="dram", bufs=2, space="DRAM") as dramp,
        ):
            # ---------------- constants -------------------------------------
            cst = constp.tile([128, 780], F32)
            mB = cst[:, 0:DP]
            mA = cst[:, 256:256 + DP]
            mT = cst[:, 512:512 + DP]
            blk = cst[:, 768:776]
            cb = cst[:, 776:780]
            # bf16 per-type parameter tables, replicated on all partitions
            ptb = constp.tile([128, 156], BF16)
            btab = ptb[:, 0:30]
            atab = ptb[:, 30:56]
            ttab = ptb[:, 56:156]
            fwork = constp.tile([128, 620], F32)
            nc.vector.memset(cb[:, 0:1], EPS)
            nc.vector.memset(cb[:, 1:2], PI / 2.0)
            nc.vector.memset(cb[:, 2:3], 1e-30)
            nc.vector.memset(cb[:, 3:4], -1.0)
            b_eps = cb[:, 0:1]
            b_pi2 = cb[:, 1:2]
            b_tiny = cb[:, 2:3]
            s_neg1 = cb[:, 3:4]

            # f32 staging of the tables (broadcast) then DVE-convert to bf16
            fb = fwork[:, 0:30]
            fa = fwork[:, 30:56]
            ft = fwork[:, 56:156]
            nc.sync.dma_start(
                out=fb,
                in_=bond_t.ap().rearrange("a b -> (a b)")[None, :].to_broadcast([128, 30]),
            )
            nc.sync.dma_start(
                out=fa,
                in_=ang_t.ap().rearrange("a b -> (a b)")[None, :].to_broadcast([128, 26]),
            )
            # torsion derived table (k, cos d, sin d, n) x 25 on one partition
            onep = fwork[0:1, 156:356]
            traw = onep[:, 0:50]
            mraw = onep[:, 50:75]
            t4 = onep[:, 75:175]
            nc.sync.dma_start(out=traw, in_=tor_t.ap().rearrange("a b -> (a b)")[None, :])
            nc.sync.dma_start(out=mraw, in_=mult_f.ap())
            t4v = t4.rearrange("p (n d) -> p n d", d=4)
            trv = traw.rearrange("p (n d) -> p n d", d=2)
            nc.vector.tensor_copy(out=t4v[:, :, 0], in_=trv[:, :, 0])                # k
            # cos d = sin(pi/2 - d); d in [0, 3.15) keeps the arg in [-pi, pi]
            nc.scalar.activation(t4v[:, :, 1], trv[:, :, 1], AF.Sin,
                                 bias=b_pi2[0:1, :], scale=s_neg1[0:1, :])
            nc.scalar.activation(t4v[:, :, 2], trv[:, :, 1], AF.Sin)                 # sin d
            nc.vector.tensor_copy(out=t4v[:, :, 3], in_=mraw)                        # n
            t4_dram = dramp.tile([1, 100], F32)
            nc.sync.dma_start(out=t4_dram[:], in_=t4)
            nc.sync.dma_start(out=ft, in_=t4_dram[:].to_broadcast([128, 100]))
            nc.vector.tensor_copy(out=btab, in_=fb)
            nc.vector.tensor_copy(out=atab, in_=fa)
            nc.vector.tensor_copy(out=ttab, in_=ft)

            # masks: with one-shot round-robin unwrap, pad list entries land on
            # partitions p%16==15 at cols u%16==15 with u >= 256 - 16*ntail
            iwork = constp.tile([128, 524], I32)
            pidx = iwork[:, 0:1]
            colx = iwork[:, 1:257]
            and15 = iwork[:, 257:258]
            r15i = iwork[:, 258:259]
            blki = iwork[:, 259:267]
            pdiv = iwork[:, 267:268]
            cm_i = iwork[:, 268:524]
            row15 = fwork[:, 356:357]
            tailf = fwork[:, 357:613]
            nc.gpsimd.iota(pidx, pattern=[[1, 1]], base=0, channel_multiplier=1)
            nc.gpsimd.iota(colx, pattern=[[1, 256]], base=0, channel_multiplier=0)
            nc.vector.tensor_scalar(out=and15, in0=pidx, scalar1=15, scalar2=None,
                                    op0=Op.bitwise_and)
            nc.vector.tensor_scalar(out=r15i, in0=and15, scalar1=15, scalar2=None,
                                    op0=Op.is_equal)
            nc.vector.tensor_copy(out=row15, in_=r15i)
            for msk, ntail in ((mB, 1), (mA, 2), (mT, 3)):
                nc.vector.tensor_scalar(out=tailf, in0=colx, scalar1=DP - 16 * ntail,
                                        scalar2=None, op0=Op.is_ge)
                nc.vector.tensor_tensor(out=msk, in0=tailf,
                                        in1=row15.to_broadcast([128, DP]), op=Op.mult)
            nc.vector.tensor_scalar(out=cm_i, in0=colx, scalar1=15, scalar2=None,
                                    op0=Op.bitwise_and)
            nc.vector.tensor_scalar(out=tailf, in0=cm_i, scalar1=15, scalar2=None,
                                    op0=Op.is_equal)
            for msk in (mB, mA, mT):
                nc.vector.tensor_tensor(out=msk, in0=msk, in1=tailf, op=Op.mult)
                nc.vector.tensor_scalar(out=msk, in0=msk, scalar1=-1.0, scalar2=1.0,
                                        op0=Op.mult, op1=Op.add)
            # selector: blk[p, c] = 1 iff p//16 == c -> PE sums each 16-part
            # group (the dedup slices are disjoint partials) into PSUM row c
            nc.vector.tensor_scalar(out=pdiv, in0=pidx, scalar1=4, scalar2=None,
                                    op0=Op.arith_shift_right)
            nc.gpsimd.iota(blki, pattern=[[1, 8]], base=0, channel_multiplier=0)
            nc.vector.tensor_tensor(out=blki, in0=pdiv.to_broadcast([128, 8]),
                                    in1=blki, op=Op.is_equal)
            nc.vector.tensor_copy(out=blk, in_=blki)

            accb = accp.tile([128, DP + 6 + 8], F32)
            scr = accb[:, 0:DP]            # TTR mandatory elementwise out
            acc6 = accb[:, DP:DP + 6]
            otmp = accb[0:8, DP + 6:DP + 12]
            rtmp = accb[:, DP + 12:DP + 13]
            opt6 = fwork[0:8, 613:619]
            nc.sync.dma_start(
                out=opt6,
                in_=opt_p.ap()[:, 0:3][:, None, :].to_broadcast([8, 2, 3]),
            )

            for w in range(2):
                # ------------- stage + column extraction -------------------
                # coords: bf16 4-wide padded atom rows; atom a = 54p + u
                coords_s = dramp.tile([8, 128 * 4 * CPP], BF16, tag="coords_s")
                bonds_s = dramp.tile([8, COLN], F32, tag="bonds_s")
                angs_s = dramp.tile([8, COLN], F32, tag="angs_s")
                tors_s = dramp.tile([8, COLN], F32, tag="tors_s")
                col_dst = [bonds_s, angs_s, tors_s]
                for s8 in range(8):
                    s = 8 * w + s8
                    stage = stagep.tile([128, LPP], F32, tag="stage")
                    nc.sync.dma_start(
                        out=stage[:], in_=feat.ap()[s].rearrange("(p f) -> p f", f=LPP)
                    )
                    st27 = stage[:].rearrange("p (u t) -> p u t", t=27)
                    cd4 = cextp.tile([128, 4 * CPP], BF16, tag="cd4")
                    nc.vector.memset(cd4[:], 0.0)
                    cd4v = cd4[:].rearrange("p (u m) -> p u m", m=4)
                    for m in range(3):
                        # coord comp m of atom 54p+u at flat 27u + 9m + 5
                        nc.vector.tensor_copy(
                            out=cd4v[:, :, m], in_=st27[:, :, 9 * m + 5])
                    nc.sync.dma_start(
                        out=coords_s[:][s8].rearrange("(p f) -> p f", f=4 * CPP),
                        in_=cd4[:],
                    )
                    stv = stage[:].rearrange("p (r n) -> p r n", n=9)
                    for k, col in enumerate((6, 7, 8)):
                        cd = cextp.tile([128, CR], F32, tag="cd")
                        nc.vector.tensor_copy(out=cd[:], in_=stv[:, :, col])
                        nc.sync.dma_start(
                            out=col_dst[k][:][s8].rearrange("(p f) -> p f", f=CR),
                            in_=cd[:],
                        )

                # ------------- gather table (replicated coords) ------------
                # partition p holds sample (p//16)'s padded bf16 coords
                table = tablep.tile([128, 4 * NATOMS], BF16, tag="table")
                nc.sync.dma_start(
                    out=table[:],
                    in_=coords_s[:][:, None, 0:4 * NATOMS].to_broadcast(
                        [8, 16, 4 * NATOMS]),
                )

                # ------------- index readback + int16 conversion -----------
                iraw = idxrawp.tile([128, 3072], F32, tag="iraw")
                braw = iraw[:, 0:768]
                araw = iraw[:, 768:1792]
                trawi = iraw[:, 1792:3072]
                nc.sync.dma_start(
                    out=braw,
                    in_=bonds_s[:][:, 0:12288].rearrange("s (j f) -> s j f", f=768))
                nc.sync.dma_start(
                    out=araw,
                    in_=angs_s[:][:, 0:16384].rearrange("s (j f) -> s j f", f=1024))
                nc.sync.dma_start(
                    out=trawi,
                    in_=tors_s[:][:, 0:20480].rearrange("s (j f) -> s j f", f=1280))

                idxt = idx16p.tile([128, 12 * 256], I16, tag="idxt")

                def idx_list(n):
                    return idxt[:, 256 * n:256 * (n + 1)]

                bv = braw.rearrange("p (e k) -> p e k", k=3)
                av = araw.rearrange("p (e k) -> p e k", k=4)
                tv = trawi.rearrange("p (e k) -> p e k", k=5)
                for k in range(3):
                    nc.vector.tensor_copy(out=idx_list(k), in_=bv[:, :, k])
                for k in range(4):
                    nc.vector.tensor_copy(out=idx_list(3 + k), in_=av[:, :, k])
                for k in range(5):
                    nc.vector.tensor_copy(out=idx_list(7 + k), in_=tv[:, :, k])
                # lists: 0,1,2 = bond i,j,t; 3..6 = angle i,j,k,t; 7..11 = tor i,j,k,l,t

                def gather_dedup(idx_n, tab_ap, n_elems, d):
                    """One-shot ap_gather of the full 4096-index list (bf16),
                    then dedup the 16x-replicated output via a partition-
                    diagonal SBUF->SBUF DMA into a dense [128, DP*d] tile."""
                    g = gathp.tile([128, LIST * d], BF16, tag="g")
                    nc.gpsimd.ap_gather(
                        out_ap=g[:].rearrange("p (n d) -> p n d", d=d),
                        in_ap=tab_ap,
                        idxs_ap=idx_list(idx_n),
                        channels=128,
                        num_elems=n_elems,
                        d=d,
                        num_idxs=LIST,
                    )
                    dn = densep.tile([128, DP * d], BF16, tag="dn")
                    nc.sync.dma_start(
                        out=dn[:],
                        in_=g[:].rearrange("(c j) f -> c j f", j=16)[:, 0, :]
                            .rearrange("c (j u) -> c j u", u=DP * d),
                    )
                    return dn

                tab4 = table[:].rearrange("p (n d) -> p n d", d=4)
                btab2 = btab.rearrange("p (n d) -> p n d", d=2)
                atab2 = atab.rearrange("p (n d) -> p n d", d=2)
                ttab4 = ttab.rearrange("p (n d) -> p n d", d=4)

                acc = acc6[:, 3 * w:3 * w + 3]
                nc.vector.memset(acc, 0.0)

                # ==================== BONDS ====================
                ci = gather_dedup(0, tab4, NATOMS, 4)
                cj = gather_dedup(1, tab4, NATOMS, 4)
                pb = gather_dedup(2, btab2, 15, 2)
                d3 = workp.tile([128, 4 * DP], F32, tag="w4a")
                nc.vector.tensor_sub(out=d3[:], in0=ci[:], in1=cj[:])
                d3s = workp.tile([128, 4 * DP], F32, tag="w4b")
                nc.vector.tensor_mul(out=d3s[:], in0=d3[:], in1=d3[:])
                wb = workp.tile([128, 8 * DP], F32, tag="w8")
                r2 = wb[:, 0:DP]
                nc.vector.tensor_reduce(
                    out=r2, in_=d3s[:].rearrange("p (n d) -> p n d", d=4),
                    axis=AX.X, op=Op.add,
                )
                r = wb[:, DP:2 * DP]
                nc.scalar.activation(r, r2, AF.Sqrt, bias=b_eps)
                pbv = pb[:].rearrange("p (n d) -> p n d", d=2)
                u = wb[:, 2 * DP:3 * DP]
                nc.vector.tensor_sub(out=u, in0=r, in1=pbv[:, :, 1])
                e = wb[:, 3 * DP:4 * DP]
                nc.scalar.activation(e, u, AF.Square)
                km = wb[:, 4 * DP:5 * DP]
                nc.vector.tensor_tensor(out=km, in0=pbv[:, :, 0], in1=mB, op=Op.mult)
                nc.vector.tensor_mul(out=scr, in0=e, in1=km)
                nc.vector.tensor_reduce(out=rtmp, in_=scr, axis=AX.X, op=Op.add)
                nc.vector.tensor_add(out=acc[:, 0:1], in0=acc[:, 0:1], in1=rtmp)

                # ==================== ANGLES ====================
                gi = gather_dedup(3, tab4, NATOMS, 4)
                gj = gather_dedup(4, tab4, NATOMS, 4)
                gk = gather_dedup(5, tab4, NATOMS, 4)
                pa = gather_dedup(6, atab2, 13, 2)
                v1 = workp.tile([128, 4 * DP], F32, tag="w4a")
                v2 = workp.tile([128, 4 * DP], F32, tag="w4b")
                nc.vector.tensor_sub(out=v1[:], in0=gi[:], in1=gj[:])
                nc.vector.tensor_sub(out=v2[:], in0=gk[:], in1=gj[:])
                prod = workp.tile([128, 4 * DP], F32, tag="w4c")
                wa = workp.tile([128, 8 * DP], F32, tag="w8")
                d11 = wa[:, 0:DP]
                d22 = wa[:, 1 * DP:2 * DP]
                d12 = wa[:, 2 * DP:3 * DP]

                def dot3(dst, a, b):
                    nc.vector.tensor_mul(out=prod[:], in0=a[:], in1=b[:])
                    nc.vector.tensor_reduce(
                        out=dst, in_=prod[:].rearrange("p (n d) -> p n d", d=4),
                        axis=AX.X, op=Op.add,
                    )

                dot3(d11, v1, v1)
                dot3(d22, v2, v2)
                dot3(d12, v1, v2)
                s1 = wa[:, 3 * DP:4 * DP]
                s2a = wa[:, 4 * DP:5 * DP]
                nc.scalar.activation(s1, d11, AF.Sqrt, bias=b_eps)
                nc.scalar.activation(s2a, d22, AF.Sqrt, bias=b_eps)
                den = wa[:, 5 * DP:6 * DP]
                nc.vector.tensor_mul(out=den, in0=s1, in1=s2a)
                cosv = wa[:, 6 * DP:7 * DP]
                nc.vector.reciprocal(out=den, in_=den)
                nc.vector.tensor_mul(out=cosv, in0=d12, in1=den)
                cosc = wa[:, 7 * DP:8 * DP]
                nc.vector.tensor_scalar(
                    out=cosc, in0=cosv, scalar1=-1.0 + 1e-6, scalar2=1.0 - 1e-6,
                    op0=Op.max, op1=Op.min,
                )
                # theta = arccos(cosc) via two bounded-arg arctan branches
                # (ACT Arctan domain is [-pi/2, pi/2] so |arg| <= 1 required):
                #  |c| >  s: theta = arctan(s/c) + pi*(c<0)
                #  |c| <= s: theta = pi/2 - arctan(c/s), s = sqrt(1-c^2)
                cc = wa[:, 0:DP]                       # d11 dead
                nc.scalar.activation(cc, cosc, AF.Square)
                om = wa[:, 1 * DP:2 * DP]              # d22 dead
                nc.vector.tensor_scalar(
                    out=om, in0=cc, scalar1=-1.0, scalar2=1.0, op0=Op.mult, op1=Op.add
                )
                sn = wa[:, 2 * DP:3 * DP]              # d12 dead
                nc.scalar.activation(sn, om, AF.Sqrt)
                sgn = wa[:, 3 * DP:4 * DP]             # s1 dead
                nc.vector.tensor_scalar(
                    out=sgn, in0=cosc, scalar1=0.0, scalar2=None, op0=Op.is_ge)
                nc.vector.tensor_scalar(
                    out=sgn, in0=sgn, scalar1=2e-18, scalar2=-1e-18,
                    op0=Op.mult, op1=Op.add)
                csafe = wa[:, 4 * DP:5 * DP]           # s2a dead
                nc.vector.tensor_add(out=csafe, in0=cosc, in1=sgn)
                ra = wa[:, 3 * DP:4 * DP]              # sgn dead
                nc.vector.reciprocal(out=csafe, in_=csafe)
                nc.vector.tensor_mul(out=ra, in0=sn, in1=csafe)
                nc.vector.tensor_scalar(
                    out=ra, in0=ra, scalar1=-1.0, scalar2=1.0, op0=Op.max, op1=Op.min)
                ata = wa[:, 4 * DP:5 * DP]             # csafe dead
                nc.scalar.activation(ata, ra, AF.Arctan)
                corr = wa[:, 5 * DP:6 * DP]            # den dead
                nc.vector.tensor_scalar(
                    out=corr, in0=cosc, scalar1=0.0, scalar2=PI, op0=Op.is_lt, op1=Op.mult
                )
                tha = wa[:, 3 * DP:4 * DP]             # ra dead
                nc.vector.tensor_add(out=tha, in0=ata, in1=corr)
                rb = wa[:, 4 * DP:5 * DP]              # ata dead
                nc.vector.reciprocal(out=sn, in_=sn)
                nc.vector.tensor_mul(out=rb, in0=cosc, in1=sn)
                nc.vector.tensor_scalar(
                    out=rb, in0=rb, scalar1=-1.0, scalar2=1.0, op0=Op.max, op1=Op.min)
                thb = wa[:, 5 * DP:6 * DP]             # corr dead
                nc.scalar.activation(thb, rb, AF.Arctan)
                nc.vector.tensor_scalar(
                    out=thb, in0=thb, scalar1=-1.0, scalar2=PI / 2.0,
                    op0=Op.mult, op1=Op.add)
                wi = workp.tile([128, 2 * DP], I32, tag="wi")
                mbr = wi[:, 0:DP]
                nc.vector.tensor_scalar(
                    out=mbr, in0=cc, scalar1=0.5, scalar2=None, op0=Op.is_gt)
                th = wa[:, 6 * DP:7 * DP]              # cosv dead
                nc.vector.select(out=th, mask=mbr, on_true=tha, on_false=thb)
                pav = pa[:].rearrange("p (n d) -> p n d", d=2)
                ua = wa[:, 0:DP]                       # cc dead
                nc.vector.tensor_sub(out=ua, in0=th, in1=pav[:, :, 1])
                ea = wa[:, 1 * DP:2 * DP]              # om dead
                nc.scalar.activation(ea, ua, AF.Square)
                kma = wa[:, 2 * DP:3 * DP]             # sn dead
                nc.vector.tensor_tensor(out=kma, in0=pav[:, :, 0], in1=mA, op=Op.mult)
                nc.vector.tensor_mul(out=scr, in0=ea, in1=kma)
                nc.vector.tensor_reduce(out=rtmp, in_=scr, axis=AX.X, op=Op.add)
                nc.vector.tensor_add(out=acc[:, 1:2], in0=acc[:, 1:2], in1=rtmp)

                # ==================== TORSIONS ====================
                ti = gather_dedup(7, tab4, NATOMS, 4)
                tj = gather_dedup(8, tab4, NATOMS, 4)
                tk_ = gather_dedup(9, tab4, NATOMS, 4)
                tl = gather_dedup(10, tab4, NATOMS, 4)
                pt = gather_dedup(11, ttab4, 25, 4)
                b1 = workp.tile([128, 4 * DP], F32, tag="w4a")
                b2 = workp.tile([128, 4 * DP], F32, tag="w4b")
                b3 = workp.tile([128, 4 * DP], F32, tag="w4c")
                nc.vector.tensor_sub(out=b1[:], in0=tj[:], in1=ti[:])
                nc.vector.tensor_sub(out=b2[:], in0=tk_[:], in1=tj[:])
                nc.vector.tensor_sub(out=b3[:], in0=tl[:], in1=tk_[:])
                pl = workp.tile([128, 9 * DP], F32, tag="w9")

                def plv(n):
                    return pl[:, DP * n:DP * (n + 1)]

                for m in range(3):
                    nc.vector.tensor_copy(
                        out=plv(0 + m),
                        in_=b1[:].rearrange("p (n d) -> p n d", d=4)[:, :, m])
                    nc.vector.tensor_copy(
                        out=plv(3 + m),
                        in_=b2[:].rearrange("p (n d) -> p n d", d=4)[:, :, m])
                    nc.vector.tensor_copy(
                        out=plv(6 + m),
                        in_=b3[:].rearrange("p (n d) -> p n d", d=4)[:, :, m])
                # n1 = b1 x b2 -> cr 0..2 ; n2 = b2 x b3 -> cr 3..5
                cr_ = workp.tile([128, 6 * DP], F32, tag="w6")

                def crv(n):
                    return cr_[:, DP * n:DP * (n + 1)]

                tmp = workp.tile([128, 2 * DP], F32, tag="w2")
                t0 = tmp[:, 0:DP]
                t1_ = tmp[:, DP:2 * DP]
                for m in range(3):
                    mp1, mp2 = (m + 1) % 3, (m + 2) % 3
                    nc.vector.tensor_mul(out=t0, in0=plv(0 + mp1), in1=plv(3 + mp2))
                    nc.vector.tensor_mul(out=t1_, in0=plv(0 + mp2), in1=plv(3 + mp1))
                    nc.vector.tensor_sub(out=crv(m), in0=t0, in1=t1_)
                    nc.vector.tensor_mul(out=t0, in0=plv(3 + mp1), in1=plv(6 + mp2))
                    nc.vector.tensor_mul(out=t1_, in0=plv(3 + mp2), in1=plv(6 + mp1))
                    nc.vector.tensor_sub(out=crv(3 + m), in0=t0, in1=t1_)
                wt = workp.tile([128, 8 * DP], F32, tag="w8")
                q2 = wt[:, 0:DP]
                nc.vector.tensor_mul(out=b1[:], in0=b2[:], in1=b2[:])  # b1 = scratch
                nc.vector.tensor_reduce(
                    out=q2, in_=b1[:].rearrange("p (n d) -> p n d", d=4),
                    axis=AX.X, op=Op.add,
                )
                # m1' = n1 x b2 (normalization folded into rn)
                mp = workp.tile([128, 4 * DP], F32, tag="w4a")

                def mpv(n):
                    return mp[:, DP * n:DP * (n + 1)]

                for m in range(3):
                    mp1, mp2 = (m + 1) % 3, (m + 2) % 3
                    nc.vector.tensor_mul(out=t0, in0=crv(mp1), in1=plv(3 + mp2))
                    nc.vector.tensor_mul(out=t1_, in0=crv(mp2), in1=plv(3 + mp1))
                    nc.vector.tensor_sub(out=mpv(m), in0=t0, in1=t1_)
                X = wt[:, 1 * DP:2 * DP]
                Y = wt[:, 2 * DP:3 * DP]
                nc.vector.tensor_mul(out=t0, in0=crv(0), in1=crv(3))
                nc.vector.tensor_mul(out=t1_, in0=crv(1), in1=crv(4))
                nc.vector.tensor_add(out=X, in0=t0, in1=t1_)
                nc.vector.tensor_mul(out=t0, in0=crv(2), in1=crv(5))
                nc.vector.tensor_add(out=X, in0=X, in1=t0)
                nc.vector.tensor_mul(out=t0, in0=mpv(0), in1=crv(3))
                nc.vector.tensor_mul(out=t1_, in0=mpv(1), in1=crv(4))
                nc.vector.tensor_add(out=Y, in0=t0, in1=t1_)
                nc.vector.tensor_mul(out=t0, in0=mpv(2), in1=crv(5))
                nc.vector.tensor_add(out=Y, in0=Y, in1=t0)
                rn = wt[:, 3 * DP:4 * DP]
                nc.scalar.activation(rn, q2, AF.Sqrt, bias=b_eps)
                y = wt[:, 4 * DP:5 * DP]
                nc.vector.reciprocal(out=rn, in_=rn)
                nc.vector.tensor_mul(out=y, in0=Y, in1=rn)
                hx = wt[:, 5 * DP:6 * DP]
                hy = wt[:, 6 * DP:7 * DP]
                nc.scalar.activation(hx, X, AF.Square)
                nc.scalar.activation(hy, y, AF.Square)
                h = wt[:, 7 * DP:8 * DP]
                nc.vector.tensor_add(out=h, in0=hx, in1=hy)
                rh = wt[:, 5 * DP:6 * DP]              # hx dead
                nc.scalar.activation(rh, h, AF.Sqrt, bias=b_tiny)
                c = wt[:, 0:DP]                        # q2 dead
                s = wt[:, 6 * DP:7 * DP]               # hy dead
                nc.vector.reciprocal(out=rh, in_=rh)
                nc.vector.tensor_mul(out=c, in0=X, in1=rh)
                nc.vector.tensor_mul(out=s, in0=y, in1=rh)
                # Chebyshev: cos/sin of 2phi and 3phi (reuse pl slices: b1/b3
                # component planes are dead after the cross products)
                cc_ = plv(0)
                c2 = plv(1)
                s2 = plv(2)
                c3 = plv(6)
                s3 = plv(7)
                sc = plv(8)
                nc.scalar.activation(cc_, c, AF.Square)
                nc.vector.tensor_scalar(
                    out=c2, in0=cc_, scalar1=2.0, scalar2=-1.0, op0=Op.mult, op1=Op.add)
                nc.vector.tensor_mul(out=sc, in0=s, in1=c)
                nc.vector.tensor_scalar(
                    out=s2, in0=sc, scalar1=2.0, scalar2=None, op0=Op.mult)
                nc.vector.tensor_scalar(
                    out=t0, in0=cc_, scalar1=4.0, scalar2=-3.0, op0=Op.mult, op1=Op.add)
                nc.vector.tensor_mul(out=c3, in0=t0, in1=c)
                nc.vector.tensor_scalar(
                    out=t0, in0=cc_, scalar1=4.0, scalar2=-1.0, op0=Op.mult, op1=Op.add)
                nc.vector.tensor_mul(out=s3, in0=t0, in1=s)
                ptv = pt[:].rearrange("p (n d) -> p n d", d=4)
                wi2 = workp.tile([128, 2 * DP], I32, tag="wi")
                m2m = wi2[:, 0:DP]
                m3m = wi2[:, DP:2 * DP]
                nc.vector.tensor_scalar(
                    out=m2m, in0=ptv[:, :, 3], scalar1=2.0, scalar2=None, op0=Op.is_equal)
                nc.vector.tensor_scalar(
                    out=m3m, in0=ptv[:, :, 3], scalar1=3.0, scalar2=None, op0=Op.is_equal)
                cn = wt[:, 3 * DP:4 * DP]              # rn dead
                sn2 = wt[:, 4 * DP:5 * DP]             # y dead
                nc.vector.select(out=cn, mask=m2m, on_true=c2, on_false=c)
                nc.vector.select(out=cn, mask=m3m, on_true=c3, on_false=cn)
                nc.vector.select(out=sn2, mask=m2m, on_true=s2, on_false=s)
                nc.vector.select(out=sn2, mask=m3m, on_true=s3, on_false=sn2)
                tt1 = wt[:, 5 * DP:6 * DP]             # rh dead
                tt2 = wt[:, 6 * DP:7 * DP]             # s dead (selects done)
                nc.vector.tensor_mul(out=tt1, in0=cn, in1=ptv[:, :, 1])
                nc.vector.tensor_mul(out=tt2, in0=sn2, in1=ptv[:, :, 2])
                esum = wt[:, 7 * DP:8 * DP]            # h dead
                nc.vector.tensor_add(out=esum, in0=tt1, in1=tt2)
                nc.vector.tensor_scalar(
                    out=esum, in0=esum, scalar1=1.0, scalar2=None, op0=Op.add)
                kmt = wt[:, 0:DP]                      # c dead
                nc.vector.tensor_tensor(out=kmt, in0=ptv[:, :, 0], in1=mT, op=Op.mult)
                nc.vector.tensor_mul(out=scr, in0=esum, in1=kmt)
                nc.vector.tensor_reduce(out=rtmp, in_=scr, axis=AX.X, op=Op.add)
                nc.vector.tensor_add(out=acc[:, 2:3], in0=acc[:, 2:3], in1=rtmp)

            # ------------- final reduction: [128, 6] -> [8, 6] -> out ------
            pacc = psump.tile([8, 6], F32, tag="pacc")
            nc.tensor.matmul(out=pacc[:], lhsT=blk, rhs=acc6, start=True, stop=True)
            nc.vector.tensor_copy(out=otmp, in_=pacc[:])
            nc.vector.tensor_mul(out=otmp, in0=otmp, in1=opt6)
            nc.sync.dma_start(out=out_d.ap()[0:8, :], in_=otmp[:, 0:3])
            nc.sync.dma_start(out=out_d.ap()[8:16, :], in_=otmp[:, 3:6])

    nc.compile()
    return nc


@functools.lru_cache(maxsize=1)
def _get_nc():
    return build_nc()


def make_in_maps(inputs):
    """Shard full inputs into 8 per-core input maps."""
    feats = np.ascontiguousarray(inputs["features"], dtype=np.float32)
    Bf = feats.shape[0]
    flat = feats.reshape(Bf, -1)
    flat = np.concatenate(
        [flat, np.zeros((Bf, FLATPAD - flat.shape[1]), np.float32)], axis=1
    )
    bond_type = np.ascontiguousarray(inputs["bond_type"], np.float32)
    angle_type = np.ascontiguousarray(inputs["angle_type"], np.float32)
    tor_type = np.ascontiguousarray(inputs["tor_type"], np.float32)
    mult_f = np.ascontiguousarray(inputs["multiplicity"], np.float32).reshape(1, 25)
    opt = np.ascontiguousarray(inputs["opt_pars"], np.float32).reshape(1, 47)
    n_nc = Bf // NS
    in_maps = []
    for k in range(n_nc):
        in_maps.append({
            "features": flat[NS * k:NS * (k + 1)],
            "bond_type": bond_type,
            "angle_type": angle_type,
            "tor_type": tor_type,
            "mult_f": mult_f,
            "opt_pars": opt,
        })
    return in_maps


def kernel(**inputs) -> np.ndarray:
    from concourse.bass_utils import run_bass_kernel_spmd

    nc = _get_nc()
    in_maps = make_in_maps(inputs)
    res = run_bass_kernel_spmd(nc, in_maps, core_ids=list(range(len(in_maps))))
    outs = [res.results[k]["out"] for k in range(len(in_maps))]
    return np.concatenate(outs, axis=0).astype(np.float32)


def simulate_one_core(inputs, nc=None):
    """CoreSim a single NC on the first 16 samples (for correctness dev)."""
    import concourse.bass_interp as bass_interp

    if nc is None:
        nc = _get_nc()
    in_map = make_in_maps(inputs)[0]
    sim = bass_interp.MultiCoreSim(nc, 1, require_finite=False, require_nnan=False)
    for name, val in in_map.items():
        sim.cores[0].tensor(name)[:] = val
    sim.simulate(check_with_hw=False)
    return np.array(sim.cores[0].mem_tensor("out"))


if __name__ == "__main__":
    nc = build_nc()
    print("build ok")


# revision 21
# speedup vs baseline: 3.2317x; 1.6426x over previous
"""Trainium2 Bass kernel for nn_LocalEnergyOpt (molecular-mechanics local energy).

Per batch sample (B=128): features[:, :, 5] packs coords [4096, 3]; col 6 bonds
(i,j,t)x4095; col 7 angles (i,j,k,t)x4094; col 8 torsions (i,j,k,l,t)x4093.
  e_bond = opt[0] * sum k_t (|ci-cj| - r0_t)^2
  e_ang  = opt[1] * sum k_t (theta - th0_t)^2, theta = arccos(clip(cos))
  e_tor  = opt[2] * sum k_t (1 + cos(n_t phi - d_t)), phi = atan2(y, x)
Output [B, 3].

Sharding: pure data parallel, 16 samples per NeuronCore across 8 cores.

Device pipeline per NC (2 waves x 8 samples; GPSIMD Q7 core c handles sample
8w+c on partitions 16c..16c+15):
  stage features flat (LPP=1458 keeps per-partition coord phase uniform) ->
  extract packed columns (strided DVE copies; coords converted to bf16 and
  re-spaced into 4-wide padded atom rows) -> dense per-sample DRAM scratch ->
  per-partition-replicated bf16 coord table + j-wrapped int16 index lists ->
  ONE ap_gather per list (num_idxs=4096): the Q7 gather ucode cost is
  num_idxs * (a + b*words), so bf16 d=4 rows (2 words) cost ~2/3 of f32
  d=3 (3 words) and one-shot drops 8x512 chunking overheads -> SBUF->SBUF
  partition-diagonal DMA dedups the 16x-replicated output into dense
  [128, 256, d] tiles -> f32 DVE/ACT energy pipeline (pad lane is zero so
  d=4 reductions are exact) -> masked reduce partials -> per-wave PE
  matmul (one-partition-per-group selector) -> [8, 6] -> scale by
  opt_pars[0:3] -> out.

One-shot gather index order is round-robin within each 16-partition group
(out position i takes the index from partition i%16, slot i//16), so pad
list entries land at dense (p%16==15, u%16==15, u >= 256-16*ntail); masks
are built for exactly those positions.

Torsion angle avoids arccos/atan2 LUTs: cos(phi), sin(phi) are formed by
normalizing (x, y) = (n1.n2, (n1 x b2).n2 / |b2|), and cos(n phi - d)
expands via Chebyshev doubling/tripling + per-type (cos d, sin d) tables.
"""

import sys
import functools

import numpy as np

sys.path.insert(0, "/opt/trn_rl_repo")

from concourse import bacc, mybir  # noqa: E402
import concourse.tile as tile  # noqa: E402
from concourse.alu_op_type import AluOpType as Op  # noqa: E402

F32 = mybir.dt.float32
BF16 = mybir.dt.bfloat16
I16 = mybir.dt.int16
I32 = mybir.dt.int32
AF = mybir.ActivationFunctionType
AX = mybir.AxisListType

# Problem constants
N_CORES = 8
NS = 16                      # samples per NeuronCore
NB, NA, NT = 4095, 4094, 4093
NATOMS = 4096
MAXLEN = 20465
LPP = 1458                   # flat f32 per partition (multiple of 27)
FLATPAD = 128 * LPP          # 186624 >= 184185
CR = LPP // 9                # 162 packed-column rows per partition
CPP = CR // 3                # 54 complete atoms per partition (coords col)
COLN = 128 * CR              # 20736 dense column length
EPS = 1e-8
PI = float(np.pi)

LIST = 4096                  # per-core index list length per class (padded)
DP = LIST // 16              # 256 dense positions per partition


def build_nc():
    nc = bacc.Bacc(None, target_bir_lowering=False, debug=False)

    feat = nc.dram_tensor("features", [NS, FLATPAD], F32, kind="ExternalInput")
    bond_t = nc.dram_tensor("bond_type", [15, 2], F32, kind="ExternalInput")
    ang_t = nc.dram_tensor("angle_type", [13, 2], F32, kind="ExternalInput")
    tor_t = nc.dram_tensor("tor_type", [25, 2], F32, kind="ExternalInput")
    mult_f = nc.dram_tensor("mult_f", [1, 25], F32, kind="ExternalInput")
    opt_p = nc.dram_tensor("opt_pars", [1, 47], F32, kind="ExternalInput")
    out_d = nc.dram_tensor("out", [NS, 3], F32, kind="ExternalOutput")

    with tile.TileContext(nc) as tc:
        with (
            tc.tile_pool(name="const", bufs=1) as constp,
            tc.tile_pool(name="stage", bufs=2) as stagep,
            tc.tile_pool(name="cext", bufs=2) as cextp,
            tc.tile_pool(name="table", bufs=1) as tablep,
            tc.tile_pool(name="idxraw", bufs=1) as idxrawp,
            tc.tile_pool(name="idx16", bufs=2) as idx16p,
            tc.tile_pool(name="gath", bufs=2) as gathp,
            tc.tile_pool(name="dense", bufs=8) as densep,
            tc.tile_pool(name="work", bufs=1) as workp,
            tc.tile_pool(name="accp", bufs=1) as accp,
            tc.tile_pool(name="psum", bufs=1, space="PSUM") as psump,
            tc.tile_pool(name="dram", bufs=2, space="DRAM") as dramp,
        ):
            # ---------------- constants -------------------------------------
            cst = constp.tile([128, 780], F32)
            mB = cst[:, 0:DP]
            mA = cst[:, 256:256 + DP]
            mT = cst[:, 512:512 + DP]
            blk = cst[:, 768:776]
            cb = cst[:, 776:780]
            fwork = constp.tile([128, 620], F32)
            nc.vector.memset(cb[:, 0:1], EPS)
            nc.vector.memset(cb[:, 1:2], PI / 2.0)
            nc.vector.memset(cb[:, 2:3], 1e-30)
            nc.vector.memset(cb[:, 3:4], -1.0)
            b_eps = cb[:, 0:1]
            b_pi2 = cb[:, 1:2]
            b_tiny = cb[:, 2:3]
            s_neg1 = cb[:, 3:4]

            # f32 staging of the tables (broadcast) then DVE-convert to bf16
            fb = fwork[:, 0:30]
            fa = fwork[:, 30:56]
            ft = fwork[:, 56:156]
            nc.sync.dma_start(
                out=fb,
                in_=bond_t.ap().rearrange("a b -> (a b)")[None, :].to_broadcast([128, 30]),
            )
            nc.sync.dma_start(
                out=fa,
                in_=ang_t.ap().rearrange("a b -> (a b)")[None, :].to_broadcast([128, 26]),
            )
            # torsion derived table (k, cos d, sin d, n) x 25 on one partition
            onep = fwork[0:1, 156:356]
            traw = onep[:, 0:50]
            mraw = onep[:, 50:75]
            t4 = onep[:, 75:175]
            nc.sync.dma_start(out=traw, in_=tor_t.ap().rearrange("a b -> (a b)")[None, :])
            nc.sync.dma_start(out=mraw, in_=mult_f.ap())
            t4v = t4.rearrange("p (n d) -> p n d", d=4)
            trv = traw.rearrange("p (n d) -> p n d", d=2)
            nc.vector.tensor_copy(out=t4v[:, :, 0], in_=trv[:, :, 0])                # k
            # cos d = sin(pi/2 - d); d in [0, 3.15) keeps the arg in [-pi, pi]
            nc.scalar.activation(t4v[:, :, 1], trv[:, :, 1], AF.Sin,
                                 bias=b_pi2[0:1, :], scale=s_neg1[0:1, :])
            nc.scalar.activation(t4v[:, :, 2], trv[:, :, 1], AF.Sin)                 # sin d
            nc.vector.tensor_copy(out=t4v[:, :, 3], in_=mraw)                        # n
            t4_dram = dramp.tile([1, 100], F32)
            nc.sync.dma_start(out=t4_dram[:], in_=t4)
            nc.sync.dma_start(out=ft, in_=t4_dram[:].to_broadcast([128, 100]))

            # masks: with one-shot round-robin unwrap, pad list entries land on
            # partitions p%16==15 at cols u%16==15 with u >= 256 - 16*ntail
            iwork = constp.tile([128, 524], I32)
            pidx = iwork[:, 0:1]
            colx = iwork[:, 1:257]
            and15 = iwork[:, 257:258]
            r15i = iwork[:, 258:259]
            blki = iwork[:, 259:267]
            pdiv = iwork[:, 267:268]
            cm_i = iwork[:, 268:524]
            row15 = fwork[:, 356:357]
            tailf = fwork[:, 357:613]
            nc.gpsimd.iota(pidx, pattern=[[1, 1]], base=0, channel_multiplier=1)
            nc.gpsimd.iota(colx, pattern=[[1, 256]], base=0, channel_multiplier=0)
            nc.vector.tensor_scalar(out=and15, in0=pidx, scalar1=15, scalar2=None,
                                    op0=Op.bitwise_and)
            nc.vector.tensor_scalar(out=r15i, in0=and15, scalar1=15, scalar2=None,
                                    op0=Op.is_equal)
            nc.vector.tensor_copy(out=row15, in_=r15i)
            for msk, ntail in ((mB, 1), (mA, 2), (mT, 3)):
                nc.vector.tensor_scalar(out=tailf, in0=colx, scalar1=DP - 16 * ntail,
                                        scalar2=None, op0=Op.is_ge)
                nc.vector.tensor_tensor(out=msk, in0=tailf,
                                        in1=row15.to_broadcast([128, DP]), op=Op.mult)
            nc.vector.tensor_scalar(out=cm_i, in0=colx, scalar1=15, scalar2=None,
                                    op0=Op.bitwise_and)
            nc.vector.tensor_scalar(out=tailf, in0=cm_i, scalar1=15, scalar2=None,
                                    op0=Op.is_equal)
            for msk in (mB, mA, mT):
                nc.vector.tensor_tensor(out=msk, in0=msk, in1=tailf, op=Op.mult)
                nc.vector.tensor_scalar(out=msk, in0=msk, scalar1=-1.0, scalar2=1.0,
                                        op0=Op.mult, op1=Op.add)
            # selector: blk[p, c] = 1 iff p//16 == c -> PE sums each 16-part
            # group (the dedup slices are disjoint partials) into PSUM row c
            nc.vector.tensor_scalar(out=pdiv, in0=pidx, scalar1=4, scalar2=None,
                                    op0=Op.arith_shift_right)
            nc.gpsimd.iota(blki, pattern=[[1, 8]], base=0, channel_multiplier=0)
            nc.vector.tensor_tensor(out=blki, in0=pdiv.to_broadcast([128, 8]),
                                    in1=blki, op=Op.is_equal)
            nc.vector.tensor_copy(out=blk, in_=blki)

            accb = accp.tile([128, DP + 6 + 8], F32)
            scr = accb[:, 0:DP]            # TTR mandatory elementwise out
            acc6 = accb[:, DP:DP + 6]
            otmp = accb[0:8, DP + 6:DP + 12]
            rtmp = accb[:, DP + 12:DP + 13]
            opt6 = fwork[0:8, 613:619]
            nc.sync.dma_start(
                out=opt6,
                in_=opt_p.ap()[:, 0:3][:, None, :].to_broadcast([8, 2, 3]),
            )

            for w in range(2):
                # ------------- stage + column extraction -------------------
                # coords: bf16 4-wide padded atom rows; atom a = 54p + u
                coords_s = dramp.tile([8, 128 * 4 * CPP], BF16, tag="coords_s")
                bonds_s = dramp.tile([8, COLN], F32, tag="bonds_s")
                angs_s = dramp.tile([8, COLN], F32, tag="angs_s")
                tors_s = dramp.tile([8, COLN], F32, tag="tors_s")
                col_dst = [bonds_s, angs_s, tors_s]
                for s8 in range(8):
                    s = 8 * w + s8
                    stage = stagep.tile([128, LPP], F32, tag="stage")
                    nc.sync.dma_start(
                        out=stage[:], in_=feat.ap()[s].rearrange("(p f) -> p f", f=LPP)
                    )
                    st27 = stage[:].rearrange("p (u t) -> p u t", t=27)
                    cd4 = cextp.tile([128, 4 * CPP], BF16, tag="cd4")
                    nc.vector.memset(cd4[:], 0.0)
                    cd4v = cd4[:].rearrange("p (u m) -> p u m", m=4)
                    for m in range(3):
                        # coord comp m of atom 54p+u at flat 27u + 9m + 5
                        nc.vector.tensor_copy(
                            out=cd4v[:, :, m], in_=st27[:, :, 9 * m + 5])
                    nc.sync.dma_start(
                        out=coords_s[:][s8].rearrange("(p f) -> p f", f=4 * CPP),
                        in_=cd4[:],
                    )
                    stv = stage[:].rearrange("p (r n) -> p r n", n=9)
                    for k, col in enumerate((6, 7, 8)):
                        cd = cextp.tile([128, CR], F32, tag="cd")
                        nc.vector.tensor_copy(out=cd[:], in_=stv[:, :, col])
                        nc.sync.dma_start(
                            out=col_dst[k][:][s8].rearrange("(p f) -> p f", f=CR),
                            in_=cd[:],
                        )

                # ------------- gather table (replicated coords) ------------
                # partition p holds sample (p//16)'s padded bf16 coords
                table = tablep.tile([128, 4 * NATOMS], BF16, tag="table")
                nc.sync.dma_start(
                    out=table[:],
                    in_=coords_s[:][:, None, 0:4 * NATOMS].to_broadcast(
                        [8, 16, 4 * NATOMS]),
                )

                # ------------- index readback + int16 conversion -----------
                iraw = idxrawp.tile([128, 3072], F32, tag="iraw")
                braw = iraw[:, 0:768]
                araw = iraw[:, 768:1792]
                trawi = iraw[:, 1792:3072]
                nc.sync.dma_start(
                    out=braw,
                    in_=bonds_s[:][:, 0:12288].rearrange("s (j f) -> s j f", f=768))
                nc.sync.dma_start(
                    out=araw,
                    in_=angs_s[:][:, 0:16384].rearrange("s (j f) -> s j f", f=1024))
                nc.sync.dma_start(
                    out=trawi,
                    in_=tors_s[:][:, 0:20480].rearrange("s (j f) -> s j f", f=1280))

                idxt = idx16p.tile([128, 9 * 256], I16, tag="idxt")

                def idx_list(n):
                    return idxt[:, 256 * n:256 * (n + 1)]

                bv = braw.rearrange("p (e k) -> p e k", k=3)
                av = araw.rearrange("p (e k) -> p e k", k=4)
                tv = trawi.rearrange("p (e k) -> p e k", k=5)
                for k in range(2):
                    nc.vector.tensor_copy(out=idx_list(k), in_=bv[:, :, k])
                for k in range(3):
                    nc.vector.tensor_copy(out=idx_list(2 + k), in_=av[:, :, k])
                for k in range(4):
                    nc.vector.tensor_copy(out=idx_list(5 + k), in_=tv[:, :, k])
                # lists: 0,1 = bond i,j; 2,3,4 = angle i,j,k; 5..8 = tor i,j,k,l

                def gather_dedup(idx_n, tab_ap, n_elems, d):
                    """One-shot ap_gather of the full 4096-index list (bf16),
                    then dedup the 16x-replicated output via a partition-
                    diagonal SBUF->SBUF DMA into a dense [128, DP*d] tile."""
                    g = gathp.tile([128, LIST * d], BF16, tag="g")
                    nc.gpsimd.ap_gather(
                        out_ap=g[:].rearrange("p (n d) -> p n d", d=d),
                        in_ap=tab_ap,
                        idxs_ap=idx_list(idx_n),
                        channels=128,
                        num_elems=n_elems,
                        d=d,
                        num_idxs=LIST,
                    )
                    dn = densep.tile([128, DP * d], BF16, tag="dn", bufs=6)
                    nc.sync.dma_start(
                        out=dn[:],
                        in_=g[:].rearrange("(c j) f -> c j f", j=16)[:, 0, :]
                            .rearrange("c (j u) -> c j u", u=DP * d),
                    )
                    return dn


                def param_select(src_view, ftab, n_types, nq):
                    """Build per-element f32 param planes from type ids via
                    DVE select-accumulate (replaces tiny-table Pool gathers).
                    ftab: f32 [128, n_types*nq] replicated table."""
                    tcol = densep.tile([128, DP], F32, tag="pf", bufs=6)
                    nc.vector.tensor_copy(out=tcol, in_=src_view)
                    msk = densep.tile([128, DP], I32, tag="pm", bufs=1)
                    outs = []
                    for q in range(nq):
                        o = densep.tile([128, DP], F32, tag="pf", bufs=6)
                        nc.vector.tensor_copy(
                            out=o, in_=ftab[:, q:q + 1].to_broadcast([128, DP]))
                        outs.append(o)
                    for t in range(1, n_types):
                        nc.vector.tensor_scalar(out=msk, in0=tcol, scalar1=float(t),
                                                scalar2=None, op0=Op.is_equal)
                        for q in range(nq):
                            nc.vector.select(
                                out=outs[q], mask=msk,
                                on_true=ftab[:, nq * t + q:nq * t + q + 1]
                                    .to_broadcast([128, DP]),
                                on_false=outs[q])
                    return outs

                tab4 = table[:].rearrange("p (n d) -> p n d", d=4)

                acc = acc6[:, 3 * w:3 * w + 3]
                nc.vector.memset(acc, 0.0)

                # ==================== BONDS ====================
                ci = gather_dedup(0, tab4, NATOMS, 4)
                cj = gather_dedup(1, tab4, NATOMS, 4)
                kb_g, r0_g = param_select(bv[:, :, 2], fb, 15, 2)
                d3 = workp.tile([128, 4 * DP], F32, tag="w4a")
                nc.vector.tensor_sub(out=d3[:], in0=ci[:], in1=cj[:])
                d3s = workp.tile([128, 4 * DP], F32, tag="w4b")
                nc.vector.tensor_mul(out=d3s[:], in0=d3[:], in1=d3[:])
                wb = workp.tile([128, 8 * DP], F32, tag="w8")
                r2 = wb[:, 0:DP]
                nc.vector.tensor_reduce(
                    out=r2, in_=d3s[:].rearrange("p (n d) -> p n d", d=4),
                    axis=AX.X, op=Op.add,
                )
                r = wb[:, DP:2 * DP]
                nc.scalar.activation(r, r2, AF.Sqrt, bias=b_eps)
                u = wb[:, 2 * DP:3 * DP]
                nc.vector.tensor_sub(out=u, in0=r, in1=r0_g[:])
                e = wb[:, 3 * DP:4 * DP]
                nc.scalar.activation(e, u, AF.Square)
                km = wb[:, 4 * DP:5 * DP]
                nc.vector.tensor_tensor(out=km, in0=kb_g[:], in1=mB, op=Op.mult)
                nc.vector.tensor_mul(out=scr, in0=e, in1=km)
                nc.vector.tensor_reduce(out=rtmp, in_=scr, axis=AX.X, op=Op.add)
                nc.vector.tensor_add(out=acc[:, 0:1], in0=acc[:, 0:1], in1=rtmp)

                # ==================== ANGLES ====================
                gi = gather_dedup(2, tab4, NATOMS, 4)
                gj = gather_dedup(3, tab4, NATOMS, 4)
                gk = gather_dedup(4, tab4, NATOMS, 4)
                ka_g, th_g = param_select(av[:, :, 3], fa, 13, 2)
                v1 = workp.tile([128, 4 * DP], F32, tag="w4a")
                v2 = workp.tile([128, 4 * DP], F32, tag="w4b")
                nc.vector.tensor_sub(out=v1[:], in0=gi[:], in1=gj[:])
                nc.vector.tensor_sub(out=v2[:], in0=gk[:], in1=gj[:])
                prod = workp.tile([128, 4 * DP], F32, tag="w4c")
                wa = workp.tile([128, 8 * DP], F32, tag="w8")
                d11 = wa[:, 0:DP]
                d22 = wa[:, 1 * DP:2 * DP]
                d12 = wa[:, 2 * DP:3 * DP]

                def dot3(dst, a, b):
                    nc.vector.tensor_mul(out=prod[:], in0=a[:], in1=b[:])
                    nc.vector.tensor_reduce(
                        out=dst, in_=prod[:].rearrange("p (n d) -> p n d", d=4),
                        axis=AX.X, op=Op.add,
                    )

                dot3(d11, v1, v1)
                dot3(d22, v2, v2)
                dot3(d12, v1, v2)
                s1 = wa[:, 3 * DP:4 * DP]
                s2a = wa[:, 4 * DP:5 * DP]
                nc.scalar.activation(s1, d11, AF.Sqrt, bias=b_eps)
                nc.scalar.activation(s2a, d22, AF.Sqrt, bias=b_eps)
                den = wa[:, 5 * DP:6 * DP]
                nc.vector.tensor_mul(out=den, in0=s1, in1=s2a)
                cosv = wa[:, 6 * DP:7 * DP]
                nc.vector.reciprocal(out=den, in_=den)
                nc.vector.tensor_mul(out=cosv, in0=d12, in1=den)
                cosc = wa[:, 7 * DP:8 * DP]
                nc.vector.tensor_scalar(
                    out=cosc, in0=cosv, scalar1=-1.0 + 1e-6, scalar2=1.0 - 1e-6,
                    op0=Op.max, op1=Op.min,
                )
                # theta = arccos(cosc) via two bounded-arg arctan branches
                # (ACT Arctan domain is [-pi/2, pi/2] so |arg| <= 1 required):
                #  |c| >  s: theta = arctan(s/c) + pi*(c<0)
                #  |c| <= s: theta = pi/2 - arctan(c/s), s = sqrt(1-c^2)
                cc = wa[:, 0:DP]                       # d11 dead
                nc.scalar.activation(cc, cosc, AF.Square)
                om = wa[:, 1 * DP:2 * DP]              # d22 dead
                nc.vector.tensor_scalar(
                    out=om, in0=cc, scalar1=-1.0, scalar2=1.0, op0=Op.mult, op1=Op.add
                )
                sn = wa[:, 2 * DP:3 * DP]              # d12 dead
                nc.scalar.activation(sn, om, AF.Sqrt)
                sgn = wa[:, 3 * DP:4 * DP]             # s1 dead
                nc.vector.tensor_scalar(
                    out=sgn, in0=cosc, scalar1=0.0, scalar2=None, op0=Op.is_ge)
                nc.vector.tensor_scalar(
                    out=sgn, in0=sgn, scalar1=2e-18, scalar2=-1e-18,
                    op0=Op.mult, op1=Op.add)
                csafe = wa[:, 4 * DP:5 * DP]           # s2a dead
                nc.vector.tensor_add(out=csafe, in0=cosc, in1=sgn)
                ra = wa[:, 3 * DP:4 * DP]              # sgn dead
                nc.vector.reciprocal(out=csafe, in_=csafe)
                nc.vector.tensor_mul(out=ra, in0=sn, in1=csafe)
                nc.vector.tensor_scalar(
                    out=ra, in0=ra, scalar1=-1.0, scalar2=1.0, op0=Op.max, op1=Op.min)
                ata = wa[:, 4 * DP:5 * DP]             # csafe dead
                nc.scalar.activation(ata, ra, AF.Arctan)
                corr = wa[:, 5 * DP:6 * DP]            # den dead
                nc.vector.tensor_scalar(
                    out=corr, in0=cosc, scalar1=0.0, scalar2=PI, op0=Op.is_lt, op1=Op.mult
                )
                tha = wa[:, 3 * DP:4 * DP]             # ra dead
                nc.vector.tensor_add(out=tha, in0=ata, in1=corr)
                rb = wa[:, 4 * DP:5 * DP]              # ata dead
                nc.vector.reciprocal(out=sn, in_=sn)
                nc.vector.tensor_mul(out=rb, in0=cosc, in1=sn)
                nc.vector.tensor_scalar(
                    out=rb, in0=rb, scalar1=-1.0, scalar2=1.0, op0=Op.max, op1=Op.min)
                thb = wa[:, 5 * DP:6 * DP]             # corr dead
                nc.scalar.activation(thb, rb, AF.Arctan)
                nc.vector.tensor_scalar(
                    out=thb, in0=thb, scalar1=-1.0, scalar2=PI / 2.0,
                    op0=Op.mult, op1=Op.add)
                wi = workp.tile([128, 2 * DP], I32, tag="wi")
                mbr = wi[:, 0:DP]
                nc.vector.tensor_scalar(
                    out=mbr, in0=cc, scalar1=0.5, scalar2=None, op0=Op.is_gt)
                th = wa[:, 6 * DP:7 * DP]              # cosv dead
                nc.vector.select(out=th, mask=mbr, on_true=tha, on_false=thb)
                ua = wa[:, 0:DP]                       # cc dead
                nc.vector.tensor_sub(out=ua, in0=th, in1=th_g[:])
                ea = wa[:, 1 * DP:2 * DP]              # om dead
                nc.scalar.activation(ea, ua, AF.Square)
                kma = wa[:, 2 * DP:3 * DP]             # sn dead
                nc.vector.tensor_tensor(out=kma, in0=ka_g[:], in1=mA, op=Op.mult)
                nc.vector.tensor_mul(out=scr, in0=ea, in1=kma)
                nc.vector.tensor_reduce(out=rtmp, in_=scr, axis=AX.X, op=Op.add)
                nc.vector.tensor_add(out=acc[:, 1:2], in0=acc[:, 1:2], in1=rtmp)

                # ==================== TORSIONS ====================
                ti = gather_dedup(5, tab4, NATOMS, 4)
                tj = gather_dedup(6, tab4, NATOMS, 4)
                tk_ = gather_dedup(7, tab4, NATOMS, 4)
                tl = gather_dedup(8, tab4, NATOMS, 4)
                kt_g, cd_g, sd_g, n_g = param_select(tv[:, :, 4], ft, 25, 4)
                b1 = workp.tile([128, 4 * DP], F32, tag="w4a")
                b2 = workp.tile([128, 4 * DP], F32, tag="w4b")
                b3 = workp.tile([128, 4 * DP], F32, tag="w4c")
                nc.vector.tensor_sub(out=b1[:], in0=tj[:], in1=ti[:])
                nc.vector.tensor_sub(out=b2[:], in0=tk_[:], in1=tj[:])
                nc.vector.tensor_sub(out=b3[:], in0=tl[:], in1=tk_[:])
                pl = workp.tile([128, 9 * DP], F32, tag="w9")

                def plv(n):
                    return pl[:, DP * n:DP * (n + 1)]

                for m in range(3):
                    nc.vector.tensor_copy(
                        out=plv(0 + m),
                        in_=b1[:].rearrange("p (n d) -> p n d", d=4)[:, :, m])
                    nc.vector.tensor_copy(
                        out=plv(3 + m),
                        in_=b2[:].rearrange("p (n d) -> p n d", d=4)[:, :, m])
                    nc.vector.tensor_copy(
                        out=plv(6 + m),
                        in_=b3[:].rearrange("p (n d) -> p n d", d=4)[:, :, m])
                # n1 = b1 x b2 -> cr 0..2 ; n2 = b2 x b3 -> cr 3..5
                cr_ = workp.tile([128, 6 * DP], F32, tag="w6")

                def crv(n):
                    return cr_[:, DP * n:DP * (n + 1)]

                tmp = workp.tile([128, 2 * DP], F32, tag="w2")
                t0 = tmp[:, 0:DP]
                t1_ = tmp[:, DP:2 * DP]
                for m in range(3):
                    mp1, mp2 = (m + 1) % 3, (m + 2) % 3
                    nc.vector.tensor_mul(out=t0, in0=plv(0 + mp1), in1=plv(3 + mp2))
                    nc.vector.tensor_mul(out=t1_, in0=plv(0 + mp2), in1=plv(3 + mp1))
                    nc.vector.tensor_sub(out=crv(m), in0=t0, in1=t1_)
                    nc.vector.tensor_mul(out=t0, in0=plv(3 + mp1), in1=plv(6 + mp2))
                    nc.vector.tensor_mul(out=t1_, in0=plv(3 + mp2), in1=plv(6 + mp1))
                    nc.vector.tensor_sub(out=crv(3 + m), in0=t0, in1=t1_)
                wt = workp.tile([128, 8 * DP], F32, tag="w8")
                q2 = wt[:, 0:DP]
                nc.vector.tensor_mul(out=b1[:], in0=b2[:], in1=b2[:])  # b1 = scratch
                nc.vector.tensor_reduce(
                    out=q2, in_=b1[:].rearrange("p (n d) -> p n d", d=4),
                    axis=AX.X, op=Op.add,
                )
                # m1' = n1 x b2 (normalization folded into rn)
                mp = workp.tile([128, 4 * DP], F32, tag="w4a")

                def mpv(n):
                    return mp[:, DP * n:DP * (n + 1)]

                for m in range(3):
                    mp1, mp2 = (m + 1) % 3, (m + 2) % 3
                    nc.vector.tensor_mul(out=t0, in0=crv(mp1), in1=plv(3 + mp2))
                    nc.vector.tensor_mul(out=t1_, in0=crv(mp2), in1=plv(3 + mp1))
                    nc.vector.tensor_sub(out=mpv(m), in0=t0, in1=t1_)
                X = wt[:, 1 * DP:2 * DP]
                Y = wt[:, 2 * DP:3 * DP]
                nc.vector.tensor_mul(out=t0, in0=crv(0), in1=crv(3))
                nc.vector.tensor_mul(out=t1_, in0=crv(1), in1=crv(4))
                nc.vector.tensor_add(out=X, in0=t0, in1=t1_)
                nc.vector.tensor_mul(out=t0, in0=crv(2), in1=crv(5))
                nc.vector.tensor_add(out=X, in0=X, in1=t0)
                nc.vector.tensor_mul(out=t0, in0=mpv(0), in1=crv(3))
                nc.vector.tensor_mul(out=t1_, in0=mpv(1), in1=crv(4))
                nc.vector.tensor_add(out=Y, in0=t0, in1=t1_)
                nc.vector.tensor_mul(out=t0, in0=mpv(2), in1=crv(5))
                nc.vector.tensor_add(out=Y, in0=Y, in1=t0)
                rn = wt[:, 3 * DP:4 * DP]
                nc.scalar.activation(rn, q2, AF.Sqrt, bias=b_eps)
                y = wt[:, 4 * DP:5 * DP]
                nc.vector.reciprocal(out=rn, in_=rn)
                nc.vector.tensor_mul(out=y, in0=Y, in1=rn)
                hx = wt[:, 5 * DP:6 * DP]
                hy = wt[:, 6 * DP:7 * DP]
                nc.scalar.activation(hx, X, AF.Square)
                nc.scalar.activation(hy, y, AF.Square)
                h = wt[:, 7 * DP:8 * DP]
                nc.vector.tensor_add(out=h, in0=hx, in1=hy)
                rh = wt[:, 5 * DP:6 * DP]              # hx dead
                nc.scalar.activation(rh, h, AF.Sqrt, bias=b_tiny)
                c = wt[:, 0:DP]                        # q2 dead
                s = wt[:, 6 * DP:7 * DP]               # hy dead
                nc.vector.reciprocal(out=rh, in_=rh)
                nc.vector.tensor_mul(out=c, in0=X, in1=rh)
                nc.vector.tensor_mul(out=s, in0=y, in1=rh)
                # Chebyshev: cos/sin of 2phi and 3phi (reuse pl slices: b1/b3
                # component planes are dead after the cross products)
                cc_ = plv(0)
                c2 = plv(1)
                s2 = plv(2)
                c3 = plv(6)
                s3 = plv(7)
                sc = plv(8)
                nc.scalar.activation(cc_, c, AF.Square)
                nc.vector.tensor_scalar(
                    out=c2, in0=cc_, scalar1=2.0, scalar2=-1.0, op0=Op.mult, op1=Op.add)
                nc.vector.tensor_mul(out=sc, in0=s, in1=c)
                nc.vector.tensor_scalar(
                    out=s2, in0=sc, scalar1=2.0, scalar2=None, op0=Op.mult)
                nc.vector.tensor_scalar(
                    out=t0, in0=cc_, scalar1=4.0, scalar2=-3.0, op0=Op.mult, op1=Op.add)
                nc.vector.tensor_mul(out=c3, in0=t0, in1=c)
                nc.vector.tensor_scalar(
                    out=t0, in0=cc_, scalar1=4.0, scalar2=-1.0, op0=Op.mult, op1=Op.add)
                nc.vector.tensor_mul(out=s3, in0=t0, in1=s)
                wi2 = workp.tile([128, 2 * DP], I32, tag="wi")
                m2m = wi2[:, 0:DP]
                m3m = wi2[:, DP:2 * DP]
                nc.vector.tensor_scalar(
                    out=m2m, in0=n_g[:], scalar1=2.0, scalar2=None, op0=Op.is_equal)
                nc.vector.tensor_scalar(
                    out=m3m, in0=n_g[:], scalar1=3.0, scalar2=None, op0=Op.is_equal)
                cn = wt[:, 3 * DP:4 * DP]              # rn dead
                sn2 = wt[:, 4 * DP:5 * DP]             # y dead
                nc.vector.select(out=cn, mask=m2m, on_true=c2, on_false=c)
                nc.vector.select(out=cn, mask=m3m, on_true=c3, on_false=cn)
                nc.vector.select(out=sn2, mask=m2m, on_true=s2, on_false=s)
                nc.vector.select(out=sn2, mask=m3m, on_true=s3, on_false=sn2)
                tt1 = wt[:, 5 * DP:6 * DP]             # rh dead
                tt2 = wt[:, 6 * DP:7 * DP]             # s dead (selects done)
                nc.vector.tensor_mul(out=tt1, in0=cn, in1=cd_g[:])
                nc.vector.tensor_mul(out=tt2, in0=sn2, in1=sd_g[:])
                esum = wt[:, 7 * DP:8 * DP]            # h dead
                nc.vector.tensor_add(out=esum, in0=tt1, in1=tt2)
                nc.vector.tensor_scalar(
                    out=esum, in0=esum, scalar1=1.0, scalar2=None, op0=Op.add)
                kmt = wt[:, 0:DP]                      # c dead
                nc.vector.tensor_tensor(out=kmt, in0=kt_g[:], in1=mT, op=Op.mult)
                nc.vector.tensor_mul(out=scr, in0=esum, in1=kmt)
                nc.vector.tensor_reduce(out=rtmp, in_=scr, axis=AX.X, op=Op.add)
                nc.vector.tensor_add(out=acc[:, 2:3], in0=acc[:, 2:3], in1=rtmp)

            # ------------- final reduction: [128, 6] -> [8, 6] -> out ------
            pacc = psump.tile([8, 6], F32, tag="pacc")
            nc.tensor.matmul(out=pacc[:], lhsT=blk, rhs=acc6, start=True, stop=True)
            nc.vector.tensor_copy(out=otmp, in_=pacc[:])
            nc.vector.tensor_mul(out=otmp, in0=otmp, in1=opt6)
            nc.sync.dma_start(out=out_d.ap()[0:8, :], in_=otmp[:, 0:3])
            nc.sync.dma_start(out=out_d.ap()[8:16, :], in_=otmp[:, 3:6])

    nc.compile()
    return nc


@functools.lru_cache(maxsize=1)
def _get_nc():
    return build_nc()


def make_in_maps(inputs):
    """Shard full inputs into 8 per-core input maps."""
    feats = np.ascontiguousarray(inputs["features"], dtype=np.float32)
    Bf = feats.shape[0]
    flat = feats.reshape(Bf, -1)
    flat = np.concatenate(
        [flat, np.zeros((Bf, FLATPAD - flat.shape[1]), np.float32)], axis=1
    )
    bond_type = np.ascontiguousarray(inputs["bond_type"], np.float32)
    angle_type = np.ascontiguousarray(inputs["angle_type"], np.float32)
    tor_type = np.ascontiguousarray(inputs["tor_type"], np.float32)
    mult_f = np.ascontiguousarray(inputs["multiplicity"], np.float32).reshape(1, 25)
    opt = np.ascontiguousarray(inputs["opt_pars"], np.float32).reshape(1, 47)
    n_nc = Bf // NS
    in_maps = []
    for k in range(n_nc):
        in_maps.append({
            "features": flat[NS * k:NS * (k + 1)],
            "bond_type": bond_type,
            "angle_type": angle_type,
            "tor_type": tor_type,
            "mult_f": mult_f,
            "opt_pars": opt,
        })
    return in_maps


def kernel(**inputs) -> np.ndarray:
    from concourse.bass_utils import run_bass_kernel_spmd

    nc = _get_nc()
    in_maps = make_in_maps(inputs)
    res = run_bass_kernel_spmd(nc, in_maps, core_ids=list(range(len(in_maps))))
    outs = [res.results[k]["out"] for k in range(len(in_maps))]
    return np.concatenate(outs, axis=0).astype(np.float32)


def simulate_one_core(inputs, nc=None):
    """CoreSim a single NC on the first 16 samples (for correctness dev)."""
    import concourse.bass_interp as bass_interp

    if nc is None:
        nc = _get_nc()
    in_map = make_in_maps(inputs)[0]
    sim = bass_interp.MultiCoreSim(nc, 1, require_finite=False, require_nnan=False)
    for name, val in in_map.items():
        sim.cores[0].tensor(name)[:] = val
    sim.simulate(check_with_hw=False)
    return np.array(sim.cores[0].mem_tensor("out"))


if __name__ == "__main__":
    nc = build_nc()
    print("build ok")


# revision 23
# speedup vs baseline: 3.2327x; 1.0003x over previous
"""Trainium2 Bass kernel for nn_LocalEnergyOpt (molecular-mechanics local energy).

Per batch sample (B=128): features[:, :, 5] packs coords [4096, 3]; col 6 bonds
(i,j,t)x4095; col 7 angles (i,j,k,t)x4094; col 8 torsions (i,j,k,l,t)x4093.
  e_bond = opt[0] * sum k_t (|ci-cj| - r0_t)^2
  e_ang  = opt[1] * sum k_t (theta - th0_t)^2, theta = arccos(clip(cos))
  e_tor  = opt[2] * sum k_t (1 + cos(n_t phi - d_t)), phi = atan2(y, x)
Output [B, 3].

Sharding: pure data parallel, 16 samples per NeuronCore across 8 cores.

Device pipeline per NC (2 waves x 8 samples; GPSIMD Q7 core c handles sample
8w+c on partitions 16c..16c+15):
  stage features flat (LPP=1458 keeps per-partition coord phase uniform) ->
  extract packed columns (strided DVE copies; coords converted to bf16 and
  re-spaced into 4-wide padded atom rows) -> dense per-sample DRAM scratch ->
  per-partition-replicated bf16 coord table + j-wrapped int16 index lists
  (9 coordinate lists; type ids stay f32) -> ONE ap_gather per coord list
  (num_idxs=4096): the Q7 gather ucode cost is num_idxs * (a + b*words),
  so bf16 d=4 rows (2 words) cost ~2/3 of f32 d=3 (3 words) and one-shot
  drops 8x512 chunking overheads -> SBUF->SBUF partition-diagonal DMA
  dedups the 16x-replicated output into dense [128, 256, 4] tiles ->
  per-type parameters built on the idle DVE via is_equal masks +
  copy_predicated accumulation from broadcast table rows (no tiny-table
  Pool gathers) -> f32 DVE/ACT energy pipeline (pad lane is zero so d=4
  reductions are exact) -> masked reduce partials -> per-wave PE matmul
  (one-partition-per-group selector) -> [8, 6] -> scale by opt_pars[0:3]
  -> out.

One-shot gather index order is round-robin within each 16-partition group
(out position i takes the index from partition i%16, slot i//16), so pad
list entries land at dense (p%16==15, u%16==15, u >= 256-16*ntail); masks
are built for exactly those positions.

Torsion angle avoids arccos/atan2 LUTs: cos(phi), sin(phi) are formed by
normalizing (x, y) = (n1.n2, (n1 x b2).n2 / |b2|), and cos(n phi - d)
expands via Chebyshev doubling/tripling + per-type (cos d, sin d) tables.
"""

import sys
import functools

import numpy as np

sys.path.insert(0, "/opt/trn_rl_repo")

from concourse import bacc, mybir  # noqa: E402
import concourse.tile as tile  # noqa: E402
from concourse.alu_op_type import AluOpType as Op  # noqa: E402

F32 = mybir.dt.float32
BF16 = mybir.dt.bfloat16
I16 = mybir.dt.int16
I32 = mybir.dt.int32
AF = mybir.ActivationFunctionType
AX = mybir.AxisListType

# Problem constants
N_CORES = 8
NS = 16                      # samples per NeuronCore
NB, NA, NT = 4095, 4094, 4093
NATOMS = 4096
MAXLEN = 20465
LPP = 1458                   # flat f32 per partition (multiple of 27)
FLATPAD = 128 * LPP          # 186624 >= 184185
CR = LPP // 9                # 162 packed-column rows per partition
CPP = CR // 3                # 54 complete atoms per partition (coords col)
COLN = 128 * CR              # 20736 dense column length
EPS = 1e-8
PI = float(np.pi)

LIST = 4096                  # per-core index list length per class (padded)
DP = LIST // 16              # 256 dense positions per partition


def build_nc():
    nc = bacc.Bacc(None, target_bir_lowering=False, debug=False)

    feat = nc.dram_tensor("features", [NS, FLATPAD], F32, kind="ExternalInput")
    bond_t = nc.dram_tensor("bond_type", [15, 2], F32, kind="ExternalInput")
    ang_t = nc.dram_tensor("angle_type", [13, 2], F32, kind="ExternalInput")
    tor_t = nc.dram_tensor("tor_type", [25, 2], F32, kind="ExternalInput")
    mult_f = nc.dram_tensor("mult_f", [1, 25], F32, kind="ExternalInput")
    opt_p = nc.dram_tensor("opt_pars", [1, 47], F32, kind="ExternalInput")
    out_d = nc.dram_tensor("out", [NS, 3], F32, kind="ExternalOutput")

    with tile.TileContext(nc) as tc:
        with (
            tc.tile_pool(name="const", bufs=1) as constp,
            tc.tile_pool(name="stage", bufs=2) as stagep,
            tc.tile_pool(name="cext", bufs=2) as cextp,
            tc.tile_pool(name="table", bufs=1) as tablep,
            tc.tile_pool(name="idxraw", bufs=1) as idxrawp,
            tc.tile_pool(name="idx16", bufs=2) as idx16p,
            tc.tile_pool(name="gath", bufs=2) as gathp,
            tc.tile_pool(name="dense", bufs=8) as densep,
            tc.tile_pool(name="work", bufs=1) as workp,
            tc.tile_pool(name="accp", bufs=1) as accp,
            tc.tile_pool(name="psum", bufs=1, space="PSUM") as psump,
            tc.tile_pool(name="dram", bufs=2, space="DRAM") as dramp,
        ):
            # ---------------- constants -------------------------------------
            cst = constp.tile([128, 780], F32)
            mB = cst[:, 0:DP]
            mA = cst[:, 256:256 + DP]
            mT = cst[:, 512:512 + DP]
            blk = cst[:, 768:776]
            cb = cst[:, 776:780]
            fwork = constp.tile([128, 620], F32)
            nc.vector.memset(cb[:, 0:1], EPS)
            nc.vector.memset(cb[:, 1:2], PI / 2.0)
            nc.vector.memset(cb[:, 2:3], 1e-30)
            nc.vector.memset(cb[:, 3:4], -1.0)
            b_eps = cb[:, 0:1]
            b_pi2 = cb[:, 1:2]
            b_tiny = cb[:, 2:3]
            s_neg1 = cb[:, 3:4]

            # f32 staging of the tables (broadcast) then DVE-convert to bf16
            fb = fwork[:, 0:30]
            fa = fwork[:, 30:56]
            ft = fwork[:, 56:156]
            nc.sync.dma_start(
                out=fb,
                in_=bond_t.ap().rearrange("a b -> (a b)")[None, :].to_broadcast([128, 30]),
            )
            nc.sync.dma_start(
                out=fa,
                in_=ang_t.ap().rearrange("a b -> (a b)")[None, :].to_broadcast([128, 26]),
            )
            # torsion derived table (k, cos d, sin d, n) x 25 on one partition
            onep = fwork[0:1, 156:356]
            traw = onep[:, 0:50]
            mraw = onep[:, 50:75]
            t4 = onep[:, 75:175]
            nc.sync.dma_start(out=traw, in_=tor_t.ap().rearrange("a b -> (a b)")[None, :])
            nc.sync.dma_start(out=mraw, in_=mult_f.ap())
            t4v = t4.rearrange("p (n d) -> p n d", d=4)
            trv = traw.rearrange("p (n d) -> p n d", d=2)
            nc.vector.tensor_copy(out=t4v[:, :, 0], in_=trv[:, :, 0])                # k
            # cos d = sin(pi/2 - d); d in [0, 3.15) keeps the arg in [-pi, pi]
            nc.scalar.activation(t4v[:, :, 1], trv[:, :, 1], AF.Sin,
                                 bias=b_pi2[0:1, :], scale=s_neg1[0:1, :])
            nc.scalar.activation(t4v[:, :, 2], trv[:, :, 1], AF.Sin)                 # sin d
            nc.vector.tensor_copy(out=t4v[:, :, 3], in_=mraw)                        # n
            t4_dram = dramp.tile([1, 100], F32)
            nc.sync.dma_start(out=t4_dram[:], in_=t4)
            nc.sync.dma_start(out=ft, in_=t4_dram[:].to_broadcast([128, 100]))

            # masks: with one-shot round-robin unwrap, pad list entries land on
            # partitions p%16==15 at cols u%16==15 with u >= 256 - 16*ntail
            iwork = constp.tile([128, 524], I32)
            pidx = iwork[:, 0:1]
            colx = iwork[:, 1:257]
            and15 = iwork[:, 257:258]
            r15i = iwork[:, 258:259]
            blki = iwork[:, 259:267]
            pdiv = iwork[:, 267:268]
            cm_i = iwork[:, 268:524]
            row15 = fwork[:, 356:357]
            tailf = fwork[:, 357:613]
            nc.gpsimd.iota(pidx, pattern=[[1, 1]], base=0, channel_multiplier=1)
            nc.gpsimd.iota(colx, pattern=[[1, 256]], base=0, channel_multiplier=0)
            nc.vector.tensor_scalar(out=and15, in0=pidx, scalar1=15, scalar2=None,
                                    op0=Op.bitwise_and)
            nc.vector.tensor_scalar(out=r15i, in0=and15, scalar1=15, scalar2=None,
                                    op0=Op.is_equal)
            nc.vector.tensor_copy(out=row15, in_=r15i)
            for msk, ntail in ((mB, 1), (mA, 2), (mT, 3)):
                nc.vector.tensor_scalar(out=tailf, in0=colx, scalar1=DP - 16 * ntail,
                                        scalar2=None, op0=Op.is_ge)
                nc.vector.tensor_tensor(out=msk, in0=tailf,
                                        in1=row15.to_broadcast([128, DP]), op=Op.mult)
            nc.vector.tensor_scalar(out=cm_i, in0=colx, scalar1=15, scalar2=None,
                                    op0=Op.bitwise_and)
            nc.vector.tensor_scalar(out=tailf, in0=cm_i, scalar1=15, scalar2=None,
                                    op0=Op.is_equal)
            for msk in (mB, mA, mT):
                nc.vector.tensor_tensor(out=msk, in0=msk, in1=tailf, op=Op.mult)
                nc.vector.tensor_scalar(out=msk, in0=msk, scalar1=-1.0, scalar2=1.0,
                                        op0=Op.mult, op1=Op.add)
            # selector: blk[p, c] = 1 iff p//16 == c -> PE sums each 16-part
            # group (the dedup slices are disjoint partials) into PSUM row c
            nc.vector.tensor_scalar(out=pdiv, in0=pidx, scalar1=4, scalar2=None,
                                    op0=Op.arith_shift_right)
            nc.gpsimd.iota(blki, pattern=[[1, 8]], base=0, channel_multiplier=0)
            nc.vector.tensor_tensor(out=blki, in0=pdiv.to_broadcast([128, 8]),
                                    in1=blki, op=Op.is_equal)
            nc.vector.tensor_copy(out=blk, in_=blki)

            accb = accp.tile([128, DP + 6 + 8], F32)
            scr = accb[:, 0:DP]            # TTR mandatory elementwise out
            acc6 = accb[:, DP:DP + 6]
            otmp = accb[0:8, DP + 6:DP + 12]
            rtmp = accb[:, DP + 12:DP + 13]
            opt6 = fwork[0:8, 613:619]
            nc.sync.dma_start(
                out=opt6,
                in_=opt_p.ap()[:, 0:3][:, None, :].to_broadcast([8, 2, 3]),
            )

            for w in range(2):
                # ------------- stage + column extraction -------------------
                # coords: bf16 4-wide padded atom rows; atom a = 54p + u
                coords_s = dramp.tile([8, 128 * 4 * CPP], BF16, tag="coords_s")
                bonds_s = dramp.tile([8, COLN], F32, tag="bonds_s")
                angs_s = dramp.tile([8, COLN], F32, tag="angs_s")
                tors_s = dramp.tile([8, COLN], F32, tag="tors_s")
                col_dst = [bonds_s, angs_s, tors_s]
                for s8 in range(8):
                    s = 8 * w + s8
                    stage = stagep.tile([128, LPP], F32, tag="stage")
                    nc.sync.dma_start(
                        out=stage[:], in_=feat.ap()[s].rearrange("(p f) -> p f", f=LPP)
                    )
                    st27 = stage[:].rearrange("p (u t) -> p u t", t=27)
                    cd4 = cextp.tile([128, 4 * CPP], BF16, tag="cd4")
                    nc.vector.memset(cd4[:], 0.0)
                    cd4v = cd4[:].rearrange("p (u m) -> p u m", m=4)
                    for m in range(3):
                        # coord comp m of atom 54p+u at flat 27u + 9m + 5
                        nc.vector.tensor_copy(
                            out=cd4v[:, :, m], in_=st27[:, :, 9 * m + 5])
                    nc.sync.dma_start(
                        out=coords_s[:][s8].rearrange("(p f) -> p f", f=4 * CPP),
                        in_=cd4[:],
                    )
                    stv = stage[:].rearrange("p (r n) -> p r n", n=9)
                    for k, col in enumerate((6, 7, 8)):
                        cd = cextp.tile([128, CR], F32, tag="cd")
                        nc.vector.tensor_copy(out=cd[:], in_=stv[:, :, col])
                        nc.sync.dma_start(
                            out=col_dst[k][:][s8].rearrange("(p f) -> p f", f=CR),
                            in_=cd[:],
                        )

                # ------------- gather table (replicated coords) ------------
                # partition p holds sample (p//16)'s padded bf16 coords
                table = tablep.tile([128, 4 * NATOMS], BF16, tag="table")
                nc.sync.dma_start(
                    out=table[:],
                    in_=coords_s[:][:, None, 0:4 * NATOMS].to_broadcast(
                        [8, 16, 4 * NATOMS]),
                )

                # ------------- index readback + int16 conversion -----------
                iraw = idxrawp.tile([128, 3072], F32, tag="iraw")
                braw = iraw[:, 0:768]
                araw = iraw[:, 768:1792]
                trawi = iraw[:, 1792:3072]
                nc.sync.dma_start(
                    out=braw,
                    in_=bonds_s[:][:, 0:12288].rearrange("s (j f) -> s j f", f=768))
                nc.sync.dma_start(
                    out=araw,
                    in_=angs_s[:][:, 0:16384].rearrange("s (j f) -> s j f", f=1024))
                nc.sync.dma_start(
                    out=trawi,
                    in_=tors_s[:][:, 0:20480].rearrange("s (j f) -> s j f", f=1280))

                idxt = idx16p.tile([128, 9 * 256], I16, tag="idxt")

                def idx_list(n):
                    return idxt[:, 256 * n:256 * (n + 1)]

                bv = braw.rearrange("p (e k) -> p e k", k=3)
                av = araw.rearrange("p (e k) -> p e k", k=4)
                tv = trawi.rearrange("p (e k) -> p e k", k=5)
                for k in range(2):
                    nc.vector.tensor_copy(out=idx_list(k), in_=bv[:, :, k])
                for k in range(3):
                    nc.vector.tensor_copy(out=idx_list(2 + k), in_=av[:, :, k])
                for k in range(4):
                    nc.vector.tensor_copy(out=idx_list(5 + k), in_=tv[:, :, k])
                # lists: 0,1 = bond i,j; 2,3,4 = angle i,j,k; 5..8 = tor i,j,k,l

                def gather_dedup(idx_n, tab_ap, n_elems, d):
                    """One-shot ap_gather of the full 4096-index list (bf16),
                    then dedup the 16x-replicated output via a partition-
                    diagonal SBUF->SBUF DMA into a dense [128, DP*d] tile."""
                    g = gathp.tile([128, LIST * d], BF16, tag="g")
                    nc.gpsimd.ap_gather(
                        out_ap=g[:].rearrange("p (n d) -> p n d", d=d),
                        in_ap=tab_ap,
                        idxs_ap=idx_list(idx_n),
                        channels=128,
                        num_elems=n_elems,
                        d=d,
                        num_idxs=LIST,
                    )
                    dn = densep.tile([128, DP * d], BF16, tag="dn", bufs=6)
                    nc.sync.dma_start(
                        out=dn[:],
                        in_=g[:].rearrange("(c j) f -> c j f", j=16)[:, 0, :]
                            .rearrange("c (j u) -> c j u", u=DP * d),
                    )
                    return dn


                def param_select(src_view, ftab, n_types, nq):
                    """Build per-element f32 param planes from type ids via
                    DVE select-accumulate (replaces tiny-table Pool gathers).
                    ftab: f32 [128, n_types*nq] replicated table."""
                    tcol = densep.tile([128, DP], F32, tag="pf", bufs=6)
                    nc.vector.tensor_copy(out=tcol, in_=src_view)
                    msk = densep.tile([128, DP], I32, tag="pm", bufs=1)
                    outs = []
                    for q in range(nq):
                        o = densep.tile([128, DP], F32, tag="pf", bufs=6)
                        nc.vector.tensor_copy(
                            out=o, in_=ftab[:, q:q + 1].to_broadcast([128, DP]))
                        outs.append(o)
                    for t in range(1, n_types):
                        nc.vector.tensor_scalar(out=msk, in0=tcol, scalar1=float(t),
                                                scalar2=None, op0=Op.is_equal)
                        for q in range(nq):
                            nc.vector.copy_predicated(
                                out=outs[q], mask=msk,
                                data=ftab[:, nq * t + q:nq * t + q + 1]
                                    .to_broadcast([128, DP]))
                    return outs

                tab4 = table[:].rearrange("p (n d) -> p n d", d=4)

                acc = acc6[:, 3 * w:3 * w + 3]
                nc.vector.memset(acc, 0.0)

                # ==================== BONDS ====================
                ci = gather_dedup(0, tab4, NATOMS, 4)
                cj = gather_dedup(1, tab4, NATOMS, 4)
                kb_g, r0_g = param_select(bv[:, :, 2], fb, 15, 2)
                d3 = workp.tile([128, 4 * DP], F32, tag="w4a")
                nc.vector.tensor_sub(out=d3[:], in0=ci[:], in1=cj[:])
                d3s = workp.tile([128, 4 * DP], F32, tag="w4b")
                nc.vector.tensor_mul(out=d3s[:], in0=d3[:], in1=d3[:])
                wb = workp.tile([128, 8 * DP], F32, tag="w8")
                r2 = wb[:, 0:DP]
                nc.vector.tensor_reduce(
                    out=r2, in_=d3s[:].rearrange("p (n d) -> p n d", d=4),
                    axis=AX.X, op=Op.add,
                )
                r = wb[:, DP:2 * DP]
                nc.scalar.activation(r, r2, AF.Sqrt, bias=b_eps)
                u = wb[:, 2 * DP:3 * DP]
                nc.vector.tensor_sub(out=u, in0=r, in1=r0_g[:])
                e = wb[:, 3 * DP:4 * DP]
                nc.scalar.activation(e, u, AF.Square)
                km = wb[:, 4 * DP:5 * DP]
                nc.vector.tensor_tensor(out=km, in0=kb_g[:], in1=mB, op=Op.mult)
                nc.vector.tensor_mul(out=scr, in0=e, in1=km)
                nc.vector.tensor_reduce(out=rtmp, in_=scr, axis=AX.X, op=Op.add)
                nc.vector.tensor_add(out=acc[:, 0:1], in0=acc[:, 0:1], in1=rtmp)

                # ==================== ANGLES ====================
                gi = gather_dedup(2, tab4, NATOMS, 4)
                gj = gather_dedup(3, tab4, NATOMS, 4)
                gk = gather_dedup(4, tab4, NATOMS, 4)
                ka_g, th_g = param_select(av[:, :, 3], fa, 13, 2)
                v1 = workp.tile([128, 4 * DP], F32, tag="w4a")
                v2 = workp.tile([128, 4 * DP], F32, tag="w4b")
                nc.vector.tensor_sub(out=v1[:], in0=gi[:], in1=gj[:])
                nc.vector.tensor_sub(out=v2[:], in0=gk[:], in1=gj[:])
                prod = workp.tile([128, 4 * DP], F32, tag="w4c")
                wa = workp.tile([128, 8 * DP], F32, tag="w8")
                d11 = wa[:, 0:DP]
                d22 = wa[:, 1 * DP:2 * DP]
                d12 = wa[:, 2 * DP:3 * DP]

                def dot3(dst, a, b):
                    nc.vector.tensor_mul(out=prod[:], in0=a[:], in1=b[:])
                    nc.vector.tensor_reduce(
                        out=dst, in_=prod[:].rearrange("p (n d) -> p n d", d=4),
                        axis=AX.X, op=Op.add,
                    )

                dot3(d11, v1, v1)
                dot3(d22, v2, v2)
                dot3(d12, v1, v2)
                s1 = wa[:, 3 * DP:4 * DP]
                s2a = wa[:, 4 * DP:5 * DP]
                nc.scalar.activation(s1, d11, AF.Sqrt, bias=b_eps)
                nc.scalar.activation(s2a, d22, AF.Sqrt, bias=b_eps)
                den = wa[:, 5 * DP:6 * DP]
                nc.vector.tensor_mul(out=den, in0=s1, in1=s2a)
                cosv = wa[:, 6 * DP:7 * DP]
                nc.vector.reciprocal(out=den, in_=den)
                nc.vector.tensor_mul(out=cosv, in0=d12, in1=den)
                cosc = wa[:, 7 * DP:8 * DP]
                nc.vector.tensor_scalar(
                    out=cosc, in0=cosv, scalar1=-1.0 + 1e-6, scalar2=1.0 - 1e-6,
                    op0=Op.max, op1=Op.min,
                )
                # theta = arccos(cosc) via two bounded-arg arctan branches
                # (ACT Arctan domain is [-pi/2, pi/2] so |arg| <= 1 required):
                #  |c| >  s: theta = arctan(s/c) + pi*(c<0)
                #  |c| <= s: theta = pi/2 - arctan(c/s), s = sqrt(1-c^2)
                cc = wa[:, 0:DP]                       # d11 dead
                nc.scalar.activation(cc, cosc, AF.Square)
                om = wa[:, 1 * DP:2 * DP]              # d22 dead
                nc.vector.tensor_scalar(
                    out=om, in0=cc, scalar1=-1.0, scalar2=1.0, op0=Op.mult, op1=Op.add
                )
                sn = wa[:, 2 * DP:3 * DP]              # d12 dead
                nc.scalar.activation(sn, om, AF.Sqrt)
                sgn = wa[:, 3 * DP:4 * DP]             # s1 dead
                nc.vector.tensor_scalar(
                    out=sgn, in0=cosc, scalar1=0.0, scalar2=None, op0=Op.is_ge)
                nc.vector.tensor_scalar(
                    out=sgn, in0=sgn, scalar1=2e-18, scalar2=-1e-18,
                    op0=Op.mult, op1=Op.add)
                csafe = wa[:, 4 * DP:5 * DP]           # s2a dead
                nc.vector.tensor_add(out=csafe, in0=cosc, in1=sgn)
                ra = wa[:, 3 * DP:4 * DP]              # sgn dead
                nc.vector.reciprocal(out=csafe, in_=csafe)
                nc.vector.tensor_mul(out=ra, in0=sn, in1=csafe)
                nc.vector.tensor_scalar(
                    out=ra, in0=ra, scalar1=-1.0, scalar2=1.0, op0=Op.max, op1=Op.min)
                ata = wa[:, 4 * DP:5 * DP]             # csafe dead
                nc.scalar.activation(ata, ra, AF.Arctan)
                corr = wa[:, 5 * DP:6 * DP]            # den dead
                nc.vector.tensor_scalar(
                    out=corr, in0=cosc, scalar1=0.0, scalar2=PI, op0=Op.is_lt, op1=Op.mult
                )
                tha = wa[:, 3 * DP:4 * DP]             # ra dead
                nc.vector.tensor_add(out=tha, in0=ata, in1=corr)
                rb = wa[:, 4 * DP:5 * DP]              # ata dead
                nc.vector.reciprocal(out=sn, in_=sn)
                nc.vector.tensor_mul(out=rb, in0=cosc, in1=sn)
                nc.vector.tensor_scalar(
                    out=rb, in0=rb, scalar1=-1.0, scalar2=1.0, op0=Op.max, op1=Op.min)
                thb = wa[:, 5 * DP:6 * DP]             # corr dead
                nc.scalar.activation(thb, rb, AF.Arctan)
                nc.vector.tensor_scalar(
                    out=thb, in0=thb, scalar1=-1.0, scalar2=PI / 2.0,
                    op0=Op.mult, op1=Op.add)
                wi = workp.tile([128, 2 * DP], I32, tag="wi")
                mbr = wi[:, 0:DP]
                nc.vector.tensor_scalar(
                    out=mbr, in0=cc, scalar1=0.5, scalar2=None, op0=Op.is_gt)
                th = wa[:, 6 * DP:7 * DP]              # cosv dead
                nc.vector.select(out=th, mask=mbr, on_true=tha, on_false=thb)
                ua = wa[:, 0:DP]                       # cc dead
                nc.vector.tensor_sub(out=ua, in0=th, in1=th_g[:])
                ea = wa[:, 1 * DP:2 * DP]              # om dead
                nc.scalar.activation(ea, ua, AF.Square)
                kma = wa[:, 2 * DP:3 * DP]             # sn dead
                nc.vector.tensor_tensor(out=kma, in0=ka_g[:], in1=mA, op=Op.mult)
                nc.vector.tensor_mul(out=scr, in0=ea, in1=kma)
                nc.vector.tensor_reduce(out=rtmp, in_=scr, axis=AX.X, op=Op.add)
                nc.vector.tensor_add(out=acc[:, 1:2], in0=acc[:, 1:2], in1=rtmp)

                # ==================== TORSIONS ====================
                ti = gather_dedup(5, tab4, NATOMS, 4)
                tj = gather_dedup(6, tab4, NATOMS, 4)
                tk_ = gather_dedup(7, tab4, NATOMS, 4)
                tl = gather_dedup(8, tab4, NATOMS, 4)
                kt_g, cd_g, sd_g, n_g = param_select(tv[:, :, 4], ft, 25, 4)
                b1 = workp.tile([128, 4 * DP], F32, tag="w4a")
                b2 = workp.tile([128, 4 * DP], F32, tag="w4b")
                b3 = workp.tile([128, 4 * DP], F32, tag="w4c")
                nc.vector.tensor_sub(out=b1[:], in0=tj[:], in1=ti[:])
                nc.vector.tensor_sub(out=b2[:], in0=tk_[:], in1=tj[:])
                nc.vector.tensor_sub(out=b3[:], in0=tl[:], in1=tk_[:])
                pl = workp.tile([128, 9 * DP], F32, tag="w9")

                def plv(n):
                    return pl[:, DP * n:DP * (n + 1)]

                for m in range(3):
                    nc.vector.tensor_copy(
                        out=plv(0 + m),
                        in_=b1[:].rearrange("p (n d) -> p n d", d=4)[:, :, m])
                    nc.vector.tensor_copy(
                        out=plv(3 + m),
                        in_=b2[:].rearrange("p (n d) -> p n d", d=4)[:, :, m])
                    nc.vector.tensor_copy(
                        out=plv(6 + m),
                        in_=b3[:].rearrange("p (n d) -> p n d", d=4)[:, :, m])
                # n1 = b1 x b2 -> cr 0..2 ; n2 = b2 x b3 -> cr 3..5
                cr_ = workp.tile([128, 6 * DP], F32, tag="w6")

                def crv(n):
                    return cr_[:, DP * n:DP * (n + 1)]

                tmp = workp.tile([128, 2 * DP], F32, tag="w2")
                t0 = tmp[:, 0:DP]
                t1_ = tmp[:, DP:2 * DP]
                for m in range(3):
                    mp1, mp2 = (m + 1) % 3, (m + 2) % 3
                    nc.vector.tensor_mul(out=t0, in0=plv(0 + mp1), in1=plv(3 + mp2))
                    nc.vector.tensor_mul(out=t1_, in0=plv(0 + mp2), in1=plv(3 + mp1))
                    nc.vector.tensor_sub(out=crv(m), in0=t0, in1=t1_)
                    nc.vector.tensor_mul(out=t0, in0=plv(3 + mp1), in1=plv(6 + mp2))
                    nc.vector.tensor_mul(out=t1_, in0=plv(3 + mp2), in1=plv(6 + mp1))
                    nc.vector.tensor_sub(out=crv(3 + m), in0=t0, in1=t1_)
                wt = workp.tile([128, 8 * DP], F32, tag="w8")
                q2 = wt[:, 0:DP]
                nc.vector.tensor_mul(out=b1[:], in0=b2[:], in1=b2[:])  # b1 = scratch
                nc.vector.tensor_reduce(
                    out=q2, in_=b1[:].rearrange("p (n d) -> p n d", d=4),
                    axis=AX.X, op=Op.add,
                )
                # m1' = n1 x b2 (normalization folded into rn)
                mp = workp.tile([128, 4 * DP], F32, tag="w4a")

                def mpv(n):
                    return mp[:, DP * n:DP * (n + 1)]

                for m in range(3):
                    mp1, mp2 = (m + 1) % 3, (m + 2) % 3
                    nc.vector.tensor_mul(out=t0, in0=crv(mp1), in1=plv(3 + mp2))
                    nc.vector.tensor_mul(out=t1_, in0=crv(mp2), in1=plv(3 + mp1))
                    nc.vector.tensor_sub(out=mpv(m), in0=t0, in1=t1_)
                X = wt[:, 1 * DP:2 * DP]
                Y = wt[:, 2 * DP:3 * DP]
                nc.vector.tensor_mul(out=t0, in0=crv(0), in1=crv(3))
                nc.vector.tensor_mul(out=t1_, in0=crv(1), in1=crv(4))
                nc.vector.tensor_add(out=X, in0=t0, in1=t1_)
                nc.vector.tensor_mul(out=t0, in0=crv(2), in1=crv(5))
                nc.vector.tensor_add(out=X, in0=X, in1=t0)
                nc.vector.tensor_mul(out=t0, in0=mpv(0), in1=crv(3))
                nc.vector.tensor_mul(out=t1_, in0=mpv(1), in1=crv(4))
                nc.vector.tensor_add(out=Y, in0=t0, in1=t1_)
                nc.vector.tensor_mul(out=t0, in0=mpv(2), in1=crv(5))
                nc.vector.tensor_add(out=Y, in0=Y, in1=t0)
                rn = wt[:, 3 * DP:4 * DP]
                nc.scalar.activation(rn, q2, AF.Sqrt, bias=b_eps)
                y = wt[:, 4 * DP:5 * DP]
                nc.vector.reciprocal(out=rn, in_=rn)
                nc.vector.tensor_mul(out=y, in0=Y, in1=rn)
                hx = wt[:, 5 * DP:6 * DP]
                hy = wt[:, 6 * DP:7 * DP]
                nc.scalar.activation(hx, X, AF.Square)
                nc.scalar.activation(hy, y, AF.Square)
                h = wt[:, 7 * DP:8 * DP]
                nc.vector.tensor_add(out=h, in0=hx, in1=hy)
                rh = wt[:, 5 * DP:6 * DP]              # hx dead
                nc.scalar.activation(rh, h, AF.Sqrt, bias=b_tiny)
                c = wt[:, 0:DP]                        # q2 dead
                s = wt[:, 6 * DP:7 * DP]               # hy dead
                nc.vector.reciprocal(out=rh, in_=rh)
                nc.vector.tensor_mul(out=c, in0=X, in1=rh)
                nc.vector.tensor_mul(out=s, in0=y, in1=rh)
                # Chebyshev: cos/sin of 2phi and 3phi (reuse pl slices: b1/b3
                # component planes are dead after the cross products)
                cc_ = plv(0)
                c2 = plv(1)
                s2 = plv(2)
                c3 = plv(6)
                s3 = plv(7)
                sc = plv(8)
                nc.scalar.activation(cc_, c, AF.Square)
                nc.vector.tensor_scalar(
                    out=c2, in0=cc_, scalar1=2.0, scalar2=-1.0, op0=Op.mult, op1=Op.add)
                nc.vector.tensor_mul(out=sc, in0=s, in1=c)
                nc.vector.tensor_scalar(
                    out=s2, in0=sc, scalar1=2.0, scalar2=None, op0=Op.mult)
                nc.vector.tensor_scalar(
                    out=t0, in0=cc_, scalar1=4.0, scalar2=-3.0, op0=Op.mult, op1=Op.add)
                nc.vector.tensor_mul(out=c3, in0=t0, in1=c)
                nc.vector.tensor_scalar(
                    out=t0, in0=cc_, scalar1=4.0, scalar2=-1.0, op0=Op.mult, op1=Op.add)
                nc.vector.tensor_mul(out=s3, in0=t0, in1=s)
                wi2 = workp.tile([128, 2 * DP], I32, tag="wi")
                m2m = wi2[:, 0:DP]
                m3m = wi2[:, DP:2 * DP]
                nc.vector.tensor_scalar(
                    out=m2m, in0=n_g[:], scalar1=2.0, scalar2=None, op0=Op.is_equal)
                nc.vector.tensor_scalar(
                    out=m3m, in0=n_g[:], scalar1=3.0, scalar2=None, op0=Op.is_equal)
                cn = wt[:, 3 * DP:4 * DP]              # rn dead
                sn2 = wt[:, 4 * DP:5 * DP]             # y dead
                nc.vector.select(out=cn, mask=m2m, on_true=c2, on_false=c)
                nc.vector.copy_predicated(out=cn, mask=m3m, data=c3)
                nc.vector.select(out=sn2, mask=m2m, on_true=s2, on_false=s)
                nc.vector.copy_predicated(out=sn2, mask=m3m, data=s3)
                tt1 = wt[:, 5 * DP:6 * DP]             # rh dead
                tt2 = wt[:, 6 * DP:7 * DP]             # s dead (selects done)
                nc.vector.tensor_mul(out=tt1, in0=cn, in1=cd_g[:])
                nc.vector.tensor_mul(out=tt2, in0=sn2, in1=sd_g[:])
                esum = wt[:, 7 * DP:8 * DP]            # h dead
                nc.vector.tensor_add(out=esum, in0=tt1, in1=tt2)
                nc.vector.tensor_scalar(
                    out=esum, in0=esum, scalar1=1.0, scalar2=None, op0=Op.add)
                kmt = wt[:, 0:DP]                      # c dead
                nc.vector.tensor_tensor(out=kmt, in0=kt_g[:], in1=mT, op=Op.mult)
                nc.vector.tensor_mul(out=scr, in0=esum, in1=kmt)
                nc.vector.tensor_reduce(out=rtmp, in_=scr, axis=AX.X, op=Op.add)
                nc.vector.tensor_add(out=acc[:, 2:3], in0=acc[:, 2:3], in1=rtmp)

            # ------------- final reduction: [128, 6] -> [8, 6] -> out ------
            pacc = psump.tile([8, 6], F32, tag="pacc")
            nc.tensor.matmul(out=pacc[:], lhsT=blk, rhs=acc6, start=True, stop=True)
            nc.vector.tensor_copy(out=otmp, in_=pacc[:])
            nc.vector.tensor_mul(out=otmp, in0=otmp, in1=opt6)
            nc.sync.dma_start(out=out_d.ap()[0:8, :], in_=otmp[:, 0:3])
            nc.sync.dma_start(out=out_d.ap()[8:16, :], in_=otmp[:, 3:6])

    nc.compile()
    return nc


@functools.lru_cache(maxsize=1)
def _get_nc():
    return build_nc()


def make_in_maps(inputs):
    """Shard full inputs into 8 per-core input maps."""
    feats = np.ascontiguousarray(inputs["features"], dtype=np.float32)
    Bf = feats.shape[0]
    flat = feats.reshape(Bf, -1)
    flat = np.concatenate(
        [flat, np.zeros((Bf, FLATPAD - flat.shape[1]), np.float32)], axis=1
    )
    bond_type = np.ascontiguousarray(inputs["bond_type"], np.float32)
    angle_type = np.ascontiguousarray(inputs["angle_type"], np.float32)
    tor_type = np.ascontiguousarray(inputs["tor_type"], np.float32)
    mult_f = np.ascontiguousarray(inputs["multiplicity"], np.float32).reshape(1, 25)
    opt = np.ascontiguousarray(inputs["opt_pars"], np.float32).reshape(1, 47)
    n_nc = Bf // NS
    in_maps = []
    for k in range(n_nc):
        in_maps.append({
            "features": flat[NS * k:NS * (k + 1)],
            "bond_type": bond_type,
            "angle_type": angle_type,
            "tor_type": tor_type,
            "mult_f": mult_f,
            "opt_pars": opt,
        })
    return in_maps


def kernel(**inputs) -> np.ndarray:
    from concourse.bass_utils import run_bass_kernel_spmd

    nc = _get_nc()
    in_maps = make_in_maps(inputs)
    res = run_bass_kernel_spmd(nc, in_maps, core_ids=list(range(len(in_maps))))
    outs = [res.results[k]["out"] for k in range(len(in_maps))]
    return np.concatenate(outs, axis=0).astype(np.float32)


def simulate_one_core(inputs, nc=None):
    """CoreSim a single NC on the first 16 samples (for correctness dev)."""
    import concourse.bass_interp as bass_interp

    if nc is None:
        nc = _get_nc()
    in_map = make_in_maps(inputs)[0]
    sim = bass_interp.MultiCoreSim(nc, 1, require_finite=False, require_nnan=False)
    for name, val in in_map.items():
        sim.cores[0].tensor(name)[:] = val
    sim.simulate(check_with_hw=False)
    return np.array(sim.cores[0].mem_tensor("out"))


if __name__ == "__main__":
    nc = build_nc()
    print("build ok")
